# revision 1
# baseline (speedup 1.0000x reference)
"""Trainium2 Bass kernel for nn_AttnBlock_61684320305872.

Computes: GroupNorm(32 groups) -> q/k/v 1x1 convs -> full self-attention over
64x64=4096 spatial positions -> output 1x1 conv -> residual add.

Sharding (8 cores): data-parallel over (batch, spatial-half). Core c handles
batch b=c//2 and query-half h=c%2. Each core computes GroupNorm + full K/V for
its batch (K/V work duplicated across the pair of cores sharing a batch) and
Q + attention rows + projection + residual for its own 2048 positions.
The host permutes each core's spatial axis so its own positions come first;
attention is invariant to key/value ordering, so no unpermute is needed on
the K/V side.

On-chip layout avoids all transposes:
  scores are built transposed  sT[j,i] = sum_d k[d,j] q[d,i]  (lhsT = k slice)
  vT[hw,c] is produced directly by the V projection (lhsT = h_ slice)
  attention out oT[c,i] = sum_j vT[j,c]^T exp_sT[j,i]  accumulates over j
  softmax denominators via a ones-column matmul; 1/den is applied after the
  output projection (it commutes: proj contracts c, den scales per-i).

Matmuls run in float32r (~1.5e-4 rel err, 4x faster than float32 on PE).
"""
import sys

sys.path.insert(0, "/opt/trn_rl_repo")

from contextlib import ExitStack

import numpy as np

import concourse.bass as bass
import concourse.tile as tile
from concourse import bacc, mybir

F32 = mybir.dt.float32
F32R = mybir.dt.float32r
AF = mybir.ActivationFunctionType
OP = mybir.AluOpType

B, C, H, W = 4, 512, 64, 64
HW = H * W            # 4096 spatial positions
OWN = HW // 2         # 2048 query positions per core
P = 128               # partitions
CO = C // P           # 4 channel chunks
BLK = 512             # spatial block width for streamed phases
NBLK = HW // BLK      # 8
NJT = HW // P         # 32 key tiles
NIC = OWN // BLK      # 4 query chunks
G = 32                # groups
GSZ = C // G          # 16 channels per group
EPS = 1e-6
SCALE = 1.0 / float(np.sqrt(C))

_CACHED_NC = None
_LAST = None


def _build():
    nc = bacc.Bacc("TRN2", target_bir_lowering=False, debug=False, num_devices=8)

    xin = nc.dram_tensor("xin", [C, HW], F32, kind="ExternalInput")
    w_d = {n: nc.dram_tensor(n, [C, C], F32, kind="ExternalInput")
           for n in ("wq", "wk", "wv", "wo")}
    # host-prepacked constants (SBUF layouts; avoids tiny-descriptor DMAs)
    vecs_d = nc.dram_tensor("vecs", [P, 20], F32, kind="ExternalInput")
    bvbc_d = nc.dram_tensor("bvbc", [P, C], F32, kind="ExternalInput")
    emat_d = nc.dram_tensor("emat2", [P, CO * G], F32, kind="ExternalInput")
    etmat_d = nc.dram_tensor("etmat", [G, C], F32, kind="ExternalInput")
    outd = nc.dram_tensor("out", [C, OWN], F32, kind="ExternalOutput")

    x_r = xin.ap().rearrange("(co p) s -> p co s", p=P)
    out_r = outd.ap().rearrange("(co p) s -> p co s", p=P)

    with tile.TileContext(nc) as tc:
        with tc.tile_pool(name="big", bufs=1) as big, \
             tc.tile_pool(name="drp", bufs=1, space="DRAM") as drp:
            # ---- long-lived state ----
            k_sb = big.tile([P, CO, HW], F32R, name="k_sb", tag="k_sb")
            vT_sb = big.tile([P, NJT, C], F32R, name="vT_sb", tag="vT_sb")
            a_sb = big.tile([P, CO], F32, name="a_sb", tag="a_sb")
            bsh_sb = big.tile([P, CO], F32, name="bsh_sb", tag="bsh_sb")
            ones_r = big.tile([P, 1], F32R, name="ones_r", tag="ones_r")
            onesrow_r = big.tile([1, P], F32R, name="onesrow_r", tag="onesrow_r")
            q_dram = drp.tile([P, CO, OWN], F32R, name="q_dram", tag="q_dram")

            vecs_sb = big.tile([P, 20], F32, name="vecs_sb", tag="vecs_sb")
            nc.sync.dma_start(out=vecs_sb, in_=vecs_d.ap())
            bq_sb, bk_sb = vecs_sb[:, 0:4], vecs_sb[:, 4:8]
            gs_sb, gb_sb = vecs_sb[:, 12:16], vecs_sb[:, 16:20]

            with ExitStack() as ph:
                # ---- phase A+B resources (released before attention) ----
                strm = ph.enter_context(tc.tile_pool(name="strm", bufs=3))
                ps1 = ph.enter_context(tc.tile_pool(name="ps1", bufs=1, space="PSUM"))

                def issue_xb(s):
                    xb = strm.tile([P, CO, BLK], F32, name=f"xb{s}", tag="xblk",
                                   bufs=2)
                    eng = nc.sync if s % 2 == 0 else nc.scalar
                    eng.dma_start(out=xb, in_=x_r[:, :, s * BLK:(s + 1) * BLK])
                    return xb

                bv_bc = strm.tile([P, C], F32, name="bv_bc", tag="bv_bc", bufs=1)

                def load_weight(n, dst, eng):
                    # staged through the hblk ring (same slot size, no extra SBUF)
                    stage = strm.tile([P, CO, C], F32, name=f"stg_{n}", tag="hblk",
                                      bufs=2)
                    eng.dma_start(out=stage,
                                  in_=w_d[n].ap().rearrange("(eo p) d -> p eo d", p=P))
                    nc.vector.tensor_copy(out=dst, in_=stage)

                # ---- phase A: GroupNorm statistics over the full batch image.
                with tc.tile_pool(name="pa", bufs=3) as pa:
                    stats_sb = pa.tile([P, CO, NBLK, 6], F32, name="stats",
                                       tag="stats", bufs=1)
                    for s in range(NBLK):
                        xb = pa.tile([P, CO, BLK], F32, name=f"xa{s}", tag="xablk")
                        eng = nc.sync if s % 2 == 0 else nc.scalar
                        eng.dma_start(out=xb, in_=x_r[:, :, s * BLK:(s + 1) * BLK])
                        for co in range(CO):
                            nc.vector.bn_stats(out=stats_sb[:, co, s, :],
                                               in_=xb[:, co, :])
                    E_sb = pa.tile([P, CO, G], F32, name="E_sb", tag="E_sb", bufs=1)
                    Et_sb = pa.tile([P, CO, P], F32, name="Et_sb", tag="Et_sb", bufs=1)
                    eps_sb = pa.tile([P, 1], F32, name="eps_sb", tag="eps_sb", bufs=1)
                    nc.vector.memset(eps_sb, EPS)
                    nc.sync.dma_start(
                        out=E_sb, in_=emat_d.ap().rearrange("p (t g) -> p t g", g=G))
                    nc.sync.dma_start(
                        out=Et_sb[:G, :, :],
                        in_=etmat_d.ap().rearrange("g (t c) -> g t c", c=P))
                    mv = pa.tile([P, CO, 2], F32, name="mv", tag="mv", bufs=1)
                    t2 = pa.tile([P, CO, 2], F32, name="t2", tag="t2", bufs=1)
                    gw = pa.tile([G, 4], F32, name="gw", tag="gw", bufs=1)
                    gsr = pa.tile([G, 2], F32, name="gsr", tag="gsr", bufs=1)
                    mrs = pa.tile([P, CO, 2], F32, name="mrs", tag="mrs", bufs=1)
                    for co in range(CO):
                        nc.vector.bn_aggr(out=mv[:, co, :], in_=stats_sb[:, co, :, :])
                    # t2 = [mean_c, var_c + mean_c^2] per channel
                    nc.vector.tensor_copy(out=t2[:, :, 0], in_=mv[:, :, 0])
                    nc.vector.tensor_mul(out=t2[:, :, 1], in0=mv[:, :, 0], in1=mv[:, :, 0])
                    nc.vector.tensor_add(out=t2[:, :, 1], in0=t2[:, :, 1], in1=mv[:, :, 1])
                    # group sums via indicator matmul -> [32, 2]
                    psg = ps1.tile([G, 2], F32, name="psg", tag="psg", bufs=1,
                                   space="PSUM")
                    for co in range(CO):
                        nc.tensor.matmul(psg, E_sb[:, co, :], t2[:, co, :],
                                         start=(co == 0), stop=(co == CO - 1))
                    # gw: [group mean, E[var+mean^2], var_g, rstd]
                    nc.scalar.activation(out=gw[:, 0:2], in_=psg, func=AF.Copy,
                                         scale=1.0 / GSZ)
                    nc.vector.tensor_mul(out=gw[:, 2:3], in0=gw[:, 0:1], in1=gw[:, 0:1])
                    nc.vector.tensor_tensor(gw[:, 2:3], gw[:, 1:2], gw[:, 2:3],
                                            OP.subtract)
                    nc.scalar.activation(out=gw[:, 3:4], in_=gw[:, 2:3], func=AF.Sqrt,
                                         bias=eps_sb[:G], scale=1.0)
                    nc.vector.reciprocal(out=gw[:, 3:4], in_=gw[:, 3:4])
                    nc.vector.tensor_copy(out=gsr[:, 0:1], in_=gw[:, 0:1])
                    nc.vector.tensor_copy(out=gsr[:, 1:2], in_=gw[:, 3:4])
                    # broadcast group (mean, rstd) back to channels
                    for co in range(CO):
                        psb = ps1.tile([P, 2], F32, name=f"psb{co}", tag="psbc", bufs=1,
                                       space="PSUM")
                        nc.tensor.matmul(psb, Et_sb[:G, co, :], gsr, start=True,
                                         stop=True)
                        nc.vector.tensor_copy(out=mrs[:, co, :], in_=psb)
                    # h = a*x + b with a = gn_scale*rstd, b = gn_bias - a*mean
                    nc.vector.tensor_mul(out=a_sb, in0=gs_sb, in1=mrs[:, :, 1])
                    nc.vector.tensor_mul(out=bsh_sb, in0=a_sb, in1=mrs[:, :, 0])
                    nc.vector.tensor_tensor(bsh_sb, gb_sb, bsh_sb, OP.subtract)

                # ---- phase B: all projections, own-half blocks first so the
                # q spill is written long before the attention phase reads it.
                ones_f = strm.tile([P, 1], F32, name="ones_f", tag="ones_f", bufs=1)
                nc.vector.memset(ones_f, 1.0)
                nc.vector.tensor_copy(out=ones_r, in_=ones_f)
                onesrow_f = strm.tile([1, P], F32, name="onesrow_f", tag="onesrow_f",
                                      bufs=1)
                nc.vector.memset(onesrow_f, 1.0)
                nc.vector.tensor_copy(out=onesrow_r, in_=onesrow_f)

                nc.sync.dma_start(out=bv_bc, in_=bvbc_d.ap())
                pw = ph.enter_context(tc.tile_pool(name="pw", bufs=1))
                wq_sb = pw.tile([P, CO, C], F32R, name="wq_sb", tag="wq")
                wk_sb = pw.tile([P, CO, C], F32R, name="wk_sb", tag="wk")
                wv_sb = pw.tile([P, CO, C], F32R, name="wv_sb", tag="wv")
                load_weight("wv", wv_sb, nc.sync)
                load_weight("wq", wq_sb, nc.scalar)
                load_weight("wk", wk_sb, nc.sync)
                for s in range(NBLK):
                    xb = issue_xb(s)
                    hb = strm.tile([P, CO, BLK], F32R, name=f"hb{s}", tag="hblk",
                                   bufs=2)
                    for co in range(CO):
                        nc.vector.tensor_scalar(hb[:, co, :], xb[:, co, :],
                                                a_sb[:, co:co + 1],
                                                bsh_sb[:, co:co + 1],
                                                OP.mult, OP.add)
                    if s < NIC:  # own query half
                        for do in range(CO):
                            psq = ps1.tile([P, BLK], F32, name=f"psq{s}_{do}",
                                           tag="psq", bufs=2, space="PSUM")
                            for eo in range(CO):
                                nc.tensor.matmul(
                                    psq, wq_sb[:, eo, do * P:(do + 1) * P],
                                    hb[:, eo, :], start=(eo == 0),
                                    stop=(eo == CO - 1))
                            qwt = strm.tile([P, BLK], F32R, name=f"qwt{s}_{do}",
                                            tag="qwt", bufs=2)
                            nc.scalar.activation(out=qwt, in_=psq, func=AF.Identity,
                                                 bias=bq_sb[:, do:do + 1], scale=1.0)
                            nc.sync.dma_start(
                                out=q_dram[:, do, s * BLK:(s + 1) * BLK], in_=qwt)
                    for do in range(CO):
                        psk = ps1.tile([P, BLK], F32, name=f"psk{s}_{do}", tag="psk",
                                       bufs=2, space="PSUM")
                        for eo in range(CO):
                            nc.tensor.matmul(psk, wk_sb[:, eo, do * P:(do + 1) * P],
                                             hb[:, eo, :], start=(eo == 0),
                                             stop=(eo == CO - 1))
                        nc.scalar.activation(out=k_sb[:, do, s * BLK:(s + 1) * BLK],
                                             in_=psk, func=AF.Identity,
                                             bias=bk_sb[:, do:do + 1], scale=1.0)
                    for it in range(BLK // P):
                        psv = ps1.tile([P, C], F32, name=f"psv{s}_{it}", tag="psv",
                                       bufs=2, space="PSUM")
                        for eo in range(CO):
                            nc.tensor.matmul(psv, hb[:, eo, it * P:(it + 1) * P],
                                             wv_sb[:, eo, :], start=(eo == 0),
                                             stop=(eo == CO - 1))
                        nc.vector.tensor_add(out=vT_sb[:, s * (BLK // P) + it, :],
                                             in0=psv, in1=bv_bc)

            # first attention q-chunk: prefetch overlaps the tail of phase B
            qsl0 = big.tile([P, CO, BLK], F32R, name="q0", tag="qsl0")
            for co in range(CO):
                eng = nc.sync if co % 2 == 0 else nc.scalar
                eng.dma_start(out=qsl0[:, co, :], in_=q_dram[:, co, 0:BLK])

            # ---- phase C: attention + projection, per 512-wide query chunk.
            # Software-pipelined emission: chunk ic's PSUM->SBUF copies are
            # emitted before chunk ic+1's score loop (freeing the pso bank
            # ring early), and its projection/epilogue after it (so the PE
            # never waits on the DVE/ACT epilogue at a chunk boundary).
            with tc.tile_pool(name="att", bufs=1) as att, \
                 tc.tile_pool(name="ps2", bufs=1, space="PSUM") as ps2:

                # wo arrives here: staged through the osc ring, rounded to f32r
                # (needed only by the first projection ~85us into phase C)
                wo_sb = att.tile([P, CO, C], F32R, name="wo_sb", tag="wo_sb")
                wo_stage = att.tile([P, CO, C], F32, name="wo_stage", tag="osc",
                                    bufs=2)
                nc.scalar.dma_start(out=wo_stage,
                                    in_=w_d["wo"].ap().rearrange("(eo p) d -> p eo d",
                                                                 p=P))
                nc.vector.tensor_copy(out=wo_sb, in_=wo_stage)

                def emit_jloop(ic, qsl):
                    pso = [ps2.tile([P, BLK], F32, name=f"pso{ic}_{ct}", tag="pso",
                                    bufs=CO, space="PSUM") for ct in range(CO)]
                    psd = ps2.tile([1, BLK], F32, name=f"psd{ic}", tag="psd", bufs=2,
                                   space="PSUM")
                    et_prev = None
                    for j in range(NJT):
                        pss = ps2.tile([P, BLK], F32, name=f"pss{ic}_{j}", tag="pss",
                                       bufs=2, space="PSUM")
                        for co in range(CO):
                            nc.tensor.matmul(pss, k_sb[:, co, j * P:(j + 1) * P],
                                             qsl[:, co, :], start=(co == 0),
                                             stop=(co == CO - 1))
                        et = att.tile([P, BLK], F32R, name=f"e{ic}_{j}", tag="exp",
                                      bufs=3)
                        nc.scalar.activation(out=et, in_=pss, func=AF.Exp, scale=SCALE)
                        for ct in range(CO):
                            nc.tensor.matmul(pso[ct], vT_sb[:, j, ct * P:(ct + 1) * P],
                                             et, start=(j == 0), stop=(j == NJT - 1))
                        if j % 2 == 0:
                            et_prev = et
                        else:
                            # tree-sum on DVE: quarters the denominator matmuls
                            es = att.tile([P, BLK], F32R, name=f"es{ic}_{j}",
                                          tag="esum", bufs=3)
                            nc.vector.tensor_add(out=es, in0=et_prev, in1=et)
                            if j % 4 == 1:
                                es_prev = es
                            else:
                                es2 = att.tile([P, BLK], F32R, name=f"es2_{ic}_{j}",
                                               tag="esum", bufs=3)
                                nc.vector.tensor_add(out=es2, in0=es_prev, in1=es)
                                nc.tensor.matmul(psd, ones_r, es2, start=(j == 3),
                                                 stop=(j == NJT - 1))
                    return pso, psd

                def emit_copies(ic, pso):
                    # free the pso bank ring: 2 copies on DVE, 2 on ScalarE
                    osc = att.tile([P, CO, BLK], F32R, name=f"osc{ic}", tag="osc",
                                   bufs=2)
                    nc.vector.tensor_copy(out=osc[:, 0, :], in_=pso[0])
                    nc.scalar.activation(out=osc[:, 1, :], in_=pso[1], func=AF.Identity)
                    nc.vector.tensor_copy(out=osc[:, 2, :], in_=pso[2])
                    nc.scalar.activation(out=osc[:, 3, :], in_=pso[3], func=AF.Identity)
                    return osc

                def emit_tail(ic, psd, osc):
                    den = att.tile([1, BLK], F32R, name=f"den{ic}", tag="den", bufs=2)
                    with nc.allow_low_precision(reason="1/den rounded to f32r for "
                                                "the broadcast matmul; ~1e-4 is "
                                                "within kernel tolerance"):
                        nc.vector.reciprocal(out=den, in_=psd)
                    # broadcast 1/den to all partitions with a K=1 matmul
                    # (SBUF APs cannot have a zero partition step, and a DRAM
                    # bounce costs ~6us of latency on the final chunk)
                    rbc_ps = ps2.tile([P, BLK], F32, name=f"rbcp{ic}", tag="psd",
                                      bufs=2, space="PSUM")
                    nc.tensor.matmul(rbc_ps, onesrow_r, den, start=True, stop=True)
                    rbc = att.tile([P, BLK], F32, name=f"rbc{ic}", tag="rbc", bufs=2)
                    nc.vector.tensor_copy(out=rbc, in_=rbc_ps)
                    for dt_ in range(CO):
                        psy = ps2.tile([P, BLK], F32, name=f"psy{ic}_{dt_}", tag="pss",
                                       bufs=2, space="PSUM")
                        for ct in range(CO):
                            nc.tensor.matmul(psy, wo_sb[:, ct, dt_ * P:(dt_ + 1) * P],
                                             osc[:, ct, :], start=(ct == 0),
                                             stop=(ct == CO - 1))
                        xr = att.tile([P, BLK], F32, name=f"xr{ic}_{dt_}", tag="xres",
                                      bufs=2)
                        nc.sync.dma_start(out=xr,
                                          in_=x_r[:, dt_, ic * BLK:(ic + 1) * BLK])
                        y = att.tile([P, BLK], F32, name=f"y{ic}_{dt_}", tag="y", bufs=2)
                        nc.vector.tensor_mul(out=y, in0=psy, in1=rbc)
                        nc.vector.tensor_scalar_add(y, y, vecs_sb[:, 8 + dt_:9 + dt_])
                        nc.vector.tensor_add(out=y, in0=y, in1=xr)
                        nc.sync.dma_start(out=out_r[:, dt_, ic * BLK:(ic + 1) * BLK],
                                          in_=y)

                def load_qsl(ic):
                    t = att.tile([P, CO, BLK], F32R, name=f"q{ic}", tag="qsl", bufs=2)
                    for co in range(CO):
                        eng = nc.sync if co % 2 == 0 else nc.scalar
                        eng.dma_start(out=t[:, co, :],
                                      in_=q_dram[:, co, ic * BLK:(ic + 1) * BLK])
                    return t

                qsl = qsl0
                prev = None
                for ic in range(NIC):
                    if prev is not None:
                        osc_p = emit_copies(prev[0], prev[1])
                    cur = (ic, *emit_jloop(ic, qsl))
                    if ic + 1 < NIC:
                        qsl = load_qsl(ic + 1)
                    if prev is not None:
                        emit_tail(prev[0], prev[2], osc_p)
                    prev = cur
                osc_p = emit_copies(prev[0], prev[1])
                emit_tail(prev[0], prev[2], osc_p)

    nc.compile()
    return nc


def _make_in_maps(inputs):
    x = np.asarray(inputs["x"], np.float32).reshape(B, C, HW)
    rep = {
        "wq": np.ascontiguousarray(np.asarray(inputs["wq"], np.float32)),
        "wk": np.ascontiguousarray(np.asarray(inputs["wk"], np.float32)),
        "wv": np.ascontiguousarray(np.asarray(inputs["wv"], np.float32)),
        "wo": np.ascontiguousarray(np.asarray(inputs["wo"], np.float32)),
        "bq": np.asarray(inputs["bq"], np.float32),
        "bk": np.asarray(inputs["bk"], np.float32),
        "bv": np.asarray(inputs["bv"], np.float32),
        "bo": np.asarray(inputs["bo"], np.float32),
        "gsc": np.asarray(inputs["gn_scale"], np.float32),
        "gbi": np.asarray(inputs["gn_bias"], np.float32),
    }
    emat = np.zeros((C, G), np.float32)
    emat[np.arange(C), np.arange(C) // GSZ] = 1.0
    # emat2[p, t*G+g] = emat[t*P+p, g]; etmat[g, t*P+c] = emat[t*P+c, g]
    rep["emat2"] = np.ascontiguousarray(
        emat.reshape(CO, P, G).transpose(1, 0, 2).reshape(P, CO * G))
    rep["etmat"] = np.ascontiguousarray(emat.T)
    vecs = np.zeros((P, 20), np.float32)
    for i, nm in enumerate(("bq", "bk", "bo", "gsc", "gbi")):
        vecs[:, 4 * i:4 * i + 4] = rep[nm].reshape(CO, P).T
    rep["vecs"] = vecs
    rep["bvbc"] = np.ascontiguousarray(np.broadcast_to(rep["bv"], (P, C)))
    for nm in ("bq", "bk", "bo", "gsc", "gbi", "bv"):
        del rep[nm]
    in_maps = []
    for core in range(8):
        b, half = core // 2, core % 2
        xb = x[b]
        own = xb[:, half * OWN:(half + 1) * OWN]
        oth = xb[:, (1 - half) * OWN:(2 - half) * OWN]
        xp = np.ascontiguousarray(np.concatenate([own, oth], axis=1))
        in_maps.append({"xin": xp, **rep})
    return in_maps


def kernel(**inputs):
    global _CACHED_NC, _LAST
    from concourse.bass_utils import run_bass_kernel_spmd

    if _CACHED_NC is None:
        _CACHED_NC = _build()
    in_maps = _make_in_maps(inputs)
    res = run_bass_kernel_spmd(_CACHED_NC, in_maps, core_ids=list(range(8)))
    _LAST = res
    out = np.empty((B, C, HW), np.float32)
    for core in range(8):
        b, half = core // 2, core % 2
        out[b][:, half * OWN:(half + 1) * OWN] = res.results[core]["out"]
    return out.reshape(B, C, H, W)



# revision 5
# speedup vs baseline: 1.2747x; 1.2747x over previous
"""Trainium2 Bass kernel for nn_AttnBlock_61684320305872.

Computes: GroupNorm(32 groups) -> q/k/v 1x1 convs -> full self-attention over
64x64=4096 spatial positions -> output 1x1 conv -> residual add.

Sharding (8 cores): data-parallel over (batch, spatial-half). Core c handles
batch b=c//2 and query-half h=c%2. Each core computes GroupNorm + full K/V for
its batch (K/V work duplicated across the pair of cores sharing a batch) and
Q + attention rows + projection + residual for its own 2048 positions.
The host permutes each core's spatial axis so its own positions come first;
attention is invariant to key/value ordering, so no unpermute is needed on
the K/V side.

On-chip layout avoids all transposes:
  scores are built transposed  sT[j,i] = sum_d k[d,j] q[d,i]  (lhsT = k slice)
  vT[hw,c] is produced directly by the V projection (lhsT = h_ slice)
  attention out oT[c,i] = sum_j vT[j,c]^T exp_sT[j,i]  accumulates over j
  softmax denominators via a ones-column matmul; 1/den is applied after the
  output projection (it commutes: proj contracts c, den scales per-i).

All heavy matmuls run in fp8e4 with DoubleRow perf mode (2 fp8 weights per PE
cell -> 256-deep contraction per pass, 2x the bf16/f32r MAC rate). Error
analysis: q/k/h/w quantization puts ~0.1 absolute error on softmax logits and
~5% relative error on the attention output, which is itself only ~3% of the
residual magnitude -> final max-abs rel err ~1e-3, far inside the 2e-2 gate.
e4m3 range management: wq/wk/wo are pre-scaled x16 on host (their 1/sqrt(C)
magnitude would land in e4m3's subnormal range) and the x16 is divided out of
the PSUM result on the way to SBUF; exp() gets a -ln8 bias (softmax is
shift-invariant) so the largest weight stays ~100x under the e4m3 max; the
attention accumulator is scaled 1/16 before requantization, with the exact
inverse folded into the softmax denominator reciprocal.

x stays resident in SBUF (read from HBM exactly once); q/k/vT live in SBUF in
fp8 (no DRAM spill); weights arrive host-prequantized in fp8.
"""
import sys

sys.path.insert(0, "/opt/trn_rl_repo")

from contextlib import ExitStack

import ml_dtypes
import numpy as np

import concourse.bass as bass
import concourse.tile as tile
from concourse import bacc, mybir

F32 = mybir.dt.float32
F32R = mybir.dt.float32r
F8 = mybir.dt.float8e4
F8NP = ml_dtypes.float8_e4m3
DR = mybir.MatmulPerfMode.DoubleRow
AF = mybir.ActivationFunctionType
OP = mybir.AluOpType

B, C, H, W = 4, 512, 64, 64
HW = H * W            # 4096 spatial positions
OWN = HW // 2         # 2048 query positions per core
P = 128               # partitions
CO = C // P           # 4 channel chunks
BLK = 512             # spatial block width for streamed phases
NBLK = HW // BLK      # 8
NJT = HW // P         # 32 key tiles
NPR = NJT // 2        # 16 key-tile pairs (DoubleRow granularity)
NIC = OWN // BLK      # 4 query chunks
G = 32                # groups
GSZ = C // G          # 16 channels per group
EPS = 1e-6
SCALE = 1.0 / float(np.sqrt(C))
WS = 16.0             # host pre-scale on wq/wk/wo (fp8 subnormal avoidance)
IWS = 1.0 / WS
EXP_BIAS = -float(np.log(8.0))  # softmax shift: keeps exp() ~100x under e4m3 max

_CACHED_NC = None
_LAST = None


def _build():
    nc = bacc.Bacc("TRN2", target_bir_lowering=False, debug=False, num_devices=8)

    xin = nc.dram_tensor("xin", [C, HW], F32, kind="ExternalInput")
    w_d = {n: nc.dram_tensor(n, [C, C], F8, kind="ExternalInput")
           for n in ("wq", "wk", "wv", "wo")}
    # host-prepacked constants (SBUF layouts; avoids tiny-descriptor DMAs)
    vecs_d = nc.dram_tensor("vecs", [P, 20], F32, kind="ExternalInput")
    bvbc_d = nc.dram_tensor("bvbc", [P, C], F32, kind="ExternalInput")
    emat_d = nc.dram_tensor("emat2", [P, CO * G], F32, kind="ExternalInput")
    etmat_d = nc.dram_tensor("etmat", [G, C], F32, kind="ExternalInput")
    outd = nc.dram_tensor("out", [C, OWN], F32, kind="ExternalOutput")

    x_r = xin.ap().rearrange("(co p) s -> p co s", p=P)
    out_r = outd.ap().rearrange("(co p) s -> p co s", p=P)

    with tile.TileContext(nc) as tc:
        with tc.tile_pool(name="big", bufs=1) as big:
            # ---- long-lived state ----
            x_sb = big.tile([P, CO, HW], F32, name="x_sb", tag="x_sb")
            k_sb = big.tile([P, CO, HW], F8, name="k_sb", tag="k_sb")
            vT_sb = big.tile([P, NJT, C], F8, name="vT_sb", tag="vT_sb")
            q_sb = big.tile([P, CO, OWN], F8, name="q_sb", tag="q_sb")
            a_sb = big.tile([P, CO], F32, name="a_sb", tag="a_sb")
            bsh_sb = big.tile([P, CO], F32, name="bsh_sb", tag="bsh_sb")
            ones_r = big.tile([P, 1], F32R, name="ones_r", tag="ones_r")
            onesrow_r = big.tile([1, P], F32R, name="onesrow_r", tag="onesrow_r")
            expb_sb = big.tile([P, 1], F32, name="expb_sb", tag="expb_sb")
            nc.vector.memset(expb_sb, EXP_BIAS)

            vecs_sb = big.tile([P, 20], F32, name="vecs_sb", tag="vecs_sb")
            nc.sync.dma_start(out=vecs_sb, in_=vecs_d.ap())
            bq_sb, bk_sb = vecs_sb[:, 0:4], vecs_sb[:, 4:8]
            gs_sb, gb_sb = vecs_sb[:, 12:16], vecs_sb[:, 16:20]

            with ExitStack() as ph:
                # ---- phase A+B resources (released before attention) ----
                strm = ph.enter_context(tc.tile_pool(name="strm", bufs=3))
                ps1 = ph.enter_context(tc.tile_pool(name="ps1", bufs=1, space="PSUM"))

                # ---- phase A: load all of x into SBUF (once) + GroupNorm
                # statistics. x DMA spread over the 3 DMA-capable queues so
                # bn_stats on DVE never blocks DMA issue.
                dma_engs = (nc.sync, nc.scalar, nc.gpsimd)
                for s in range(NBLK):
                    dma_engs[s % 3].dma_start(
                        out=x_sb[:, :, s * BLK:(s + 1) * BLK],
                        in_=x_r[:, :, s * BLK:(s + 1) * BLK])
                with tc.tile_pool(name="pa", bufs=3) as pa:
                    stats_sb = pa.tile([P, CO, NBLK, 6], F32, name="stats",
                                       tag="stats", bufs=1)
                    for s in range(NBLK):
                        for co in range(CO):
                            nc.vector.bn_stats(
                                out=stats_sb[:, co, s, :],
                                in_=x_sb[:, co, s * BLK:(s + 1) * BLK])
                    E_sb = pa.tile([P, CO, G], F32, name="E_sb", tag="E_sb", bufs=1)
                    Et_sb = pa.tile([P, CO, P], F32, name="Et_sb", tag="Et_sb", bufs=1)
                    eps_sb = pa.tile([P, 1], F32, name="eps_sb", tag="eps_sb", bufs=1)
                    nc.vector.memset(eps_sb, EPS)
                    nc.sync.dma_start(
                        out=E_sb, in_=emat_d.ap().rearrange("p (t g) -> p t g", g=G))
                    nc.sync.dma_start(
                        out=Et_sb[:G, :, :],
                        in_=etmat_d.ap().rearrange("g (t c) -> g t c", c=P))
                    mv = pa.tile([P, CO, 2], F32, name="mv", tag="mv", bufs=1)
                    t2 = pa.tile([P, CO, 2], F32, name="t2", tag="t2", bufs=1)
                    gw = pa.tile([G, 4], F32, name="gw", tag="gw", bufs=1)
                    gsr = pa.tile([G, 2], F32, name="gsr", tag="gsr", bufs=1)
                    mrs = pa.tile([P, CO, 2], F32, name="mrs", tag="mrs", bufs=1)
                    for co in range(CO):
                        nc.vector.bn_aggr(out=mv[:, co, :], in_=stats_sb[:, co, :, :])
                    # t2 = [mean_c, var_c + mean_c^2] per channel
                    nc.vector.tensor_copy(out=t2[:, :, 0], in_=mv[:, :, 0])
                    nc.vector.tensor_mul(out=t2[:, :, 1], in0=mv[:, :, 0], in1=mv[:, :, 0])
                    nc.vector.tensor_add(out=t2[:, :, 1], in0=t2[:, :, 1], in1=mv[:, :, 1])
                    # group sums via indicator matmul -> [32, 2]
                    psg = ps1.tile([G, 2], F32, name="psg", tag="psg", bufs=1,
                                   space="PSUM")
                    for co in range(CO):
                        nc.tensor.matmul(psg, E_sb[:, co, :], t2[:, co, :],
                                         start=(co == 0), stop=(co == CO - 1))
                    # gw: [group mean, E[var+mean^2], var_g, rstd]
                    nc.scalar.activation(out=gw[:, 0:2], in_=psg, func=AF.Copy,
                                         scale=1.0 / GSZ)
                    nc.vector.tensor_mul(out=gw[:, 2:3], in0=gw[:, 0:1], in1=gw[:, 0:1])
                    nc.vector.tensor_tensor(gw[:, 2:3], gw[:, 1:2], gw[:, 2:3],
                                            OP.subtract)
                    nc.scalar.activation(out=gw[:, 3:4], in_=gw[:, 2:3], func=AF.Sqrt,
                                         bias=eps_sb[:G], scale=1.0)
                    nc.vector.reciprocal(out=gw[:, 3:4], in_=gw[:, 3:4])
                    nc.vector.tensor_copy(out=gsr[:, 0:1], in_=gw[:, 0:1])
                    nc.vector.tensor_copy(out=gsr[:, 1:2], in_=gw[:, 3:4])
                    # broadcast group (mean, rstd) back to channels
                    for co in range(CO):
                        psb = ps1.tile([P, 2], F32, name=f"psb{co}", tag="psbc", bufs=1,
                                       space="PSUM")
                        nc.tensor.matmul(psb, Et_sb[:G, co, :], gsr, start=True,
                                         stop=True)
                        nc.vector.tensor_copy(out=mrs[:, co, :], in_=psb)
                    # h = a*x + b with a = gn_scale*rstd, b = gn_bias - a*mean
                    nc.vector.tensor_mul(out=a_sb, in0=gs_sb, in1=mrs[:, :, 1])
                    nc.vector.tensor_mul(out=bsh_sb, in0=a_sb, in1=mrs[:, :, 0])
                    nc.vector.tensor_tensor(bsh_sb, gb_sb, bsh_sb, OP.subtract)

                # ---- phase B: all projections (fp8 DoubleRow), own-half
                # blocks first so q is ready long before attention reads it.
                ones_f = strm.tile([P, 1], F32, name="ones_f", tag="ones_f", bufs=1)
                nc.vector.memset(ones_f, 1.0)
                nc.vector.tensor_copy(out=ones_r, in_=ones_f)
                onesrow_f = strm.tile([1, P], F32, name="onesrow_f", tag="onesrow_f",
                                      bufs=1)
                nc.vector.memset(onesrow_f, 1.0)
                nc.vector.tensor_copy(out=onesrow_r, in_=onesrow_f)

                bv_bc = strm.tile([P, C], F32, name="bv_bc", tag="bv_bc", bufs=1)
                nc.sync.dma_start(out=bv_bc, in_=bvbc_d.ap())
                pw = ph.enter_context(tc.tile_pool(name="pw", bufs=1))
                wq_sb = pw.tile([P, CO, C], F8, name="wq_sb", tag="wq")
                wk_sb = pw.tile([P, CO, C], F8, name="wk_sb", tag="wk")
                wv_sb = pw.tile([P, CO, C], F8, name="wv_sb", tag="wv")
                for nm, dst, eng in (("wv", wv_sb, nc.sync),
                                     ("wq", wq_sb, nc.scalar),
                                     ("wk", wk_sb, nc.sync)):
                    eng.dma_start(out=dst,
                                  in_=w_d[nm].ap().rearrange("(eo p) d -> p eo d", p=P))
                for s in range(NBLK):
                    hb = strm.tile([P, CO, BLK], F8, name=f"hb{s}", tag="hblk",
                                   bufs=2)
                    for co in range(CO):
                        nc.vector.tensor_scalar(hb[:, co, :],
                                                x_sb[:, co, s * BLK:(s + 1) * BLK],
                                                a_sb[:, co:co + 1],
                                                bsh_sb[:, co:co + 1],
                                                OP.mult, OP.add)
                    if s < NIC:  # own query half
                        for do in range(CO):
                            psq = ps1.tile([P, BLK], F32, name=f"psq{s}_{do}",
                                           tag="psq", bufs=2, space="PSUM")
                            for t in range(2):
                                nc.tensor.matmul(
                                    psq, wq_sb[:, 2 * t:2 * t + 2, do * P:(do + 1) * P],
                                    hb[:, 2 * t:2 * t + 2, :], start=(t == 0),
                                    stop=(t == 1), perf_mode=DR)
                            nc.vector.tensor_scalar(
                                q_sb[:, do, s * BLK:(s + 1) * BLK], psq,
                                IWS, bq_sb[:, do:do + 1], OP.mult, OP.add)
                    for do in range(CO):
                        psk = ps1.tile([P, BLK], F32, name=f"psk{s}_{do}", tag="psk",
                                       bufs=2, space="PSUM")
                        for t in range(2):
                            nc.tensor.matmul(
                                psk, wk_sb[:, 2 * t:2 * t + 2, do * P:(do + 1) * P],
                                hb[:, 2 * t:2 * t + 2, :], start=(t == 0),
                                stop=(t == 1), perf_mode=DR)
                        nc.scalar.activation(out=k_sb[:, do, s * BLK:(s + 1) * BLK],
                                             in_=psk, func=AF.Identity,
                                             bias=bk_sb[:, do:do + 1], scale=IWS)
                    for it in range(BLK // P):
                        psv = ps1.tile([P, C], F32, name=f"psv{s}_{it}", tag="psv",
                                       bufs=2, space="PSUM")
                        for t in range(2):
                            nc.tensor.matmul(
                                psv, hb[:, 2 * t:2 * t + 2, it * P:(it + 1) * P],
                                wv_sb[:, 2 * t:2 * t + 2, :], start=(t == 0),
                                stop=(t == 1), perf_mode=DR)
                        nc.vector.tensor_add(out=vT_sb[:, s * (BLK // P) + it, :],
                                             in0=psv, in1=bv_bc)

            # ---- phase C: attention + projection, per 512-wide query chunk.
            # Software-pipelined emission: chunk ic's PSUM->SBUF copies are
            # emitted before chunk ic+1's score loop (freeing the pso bank
            # ring early), and its projection/epilogue after it (so the PE
            # never waits on the DVE/ACT epilogue at a chunk boundary).
            with tc.tile_pool(name="att", bufs=1) as att, \
                 tc.tile_pool(name="ps2", bufs=1, space="PSUM") as ps2:

                wo_sb = att.tile([P, CO, C], F8, name="wo_sb", tag="wo_sb")
                nc.scalar.dma_start(out=wo_sb,
                                    in_=w_d["wo"].ap().rearrange("(eo p) d -> p eo d",
                                                                 p=P))

                def emit_jloop(ic):
                    pso = [ps2.tile([P, BLK], F32, name=f"pso{ic}_{ct}", tag="pso",
                                    bufs=CO, space="PSUM") for ct in range(CO)]
                    psd = ps2.tile([1, BLK], F32, name=f"psd{ic}", tag="psd", bufs=2,
                                   space="PSUM")
                    es_prev = None
                    for jp in range(NPR):
                        ep = att.tile([P, 2, BLK], F8, name=f"ep{ic}_{jp}", tag="ep",
                                      bufs=3)
                        for jj in range(2):
                            j = 2 * jp + jj
                            pss = ps2.tile([P, BLK], F32, name=f"pss{ic}_{j}",
                                           tag="pss", bufs=2, space="PSUM")
                            for t in range(2):
                                nc.tensor.matmul(
                                    pss, k_sb[:, 2 * t:2 * t + 2, j * P:(j + 1) * P],
                                    q_sb[:, 2 * t:2 * t + 2,
                                         ic * BLK:(ic + 1) * BLK],
                                    start=(t == 0), stop=(t == 1), perf_mode=DR)
                            nc.scalar.activation(out=ep[:, jj, :], in_=pss,
                                                 func=AF.Exp, bias=expb_sb,
                                                 scale=SCALE)
                        for ct in range(CO):
                            nc.tensor.matmul(
                                pso[ct],
                                vT_sb[:, 2 * jp:2 * jp + 2, ct * P:(ct + 1) * P],
                                ep, start=(jp == 0), stop=(jp == NPR - 1),
                                perf_mode=DR)
                        # denominator: pairwise DVE sums, then a ones-matmul
                        es = att.tile([P, BLK], F32R, name=f"es{ic}_{jp}",
                                      tag="esum", bufs=3)
                        nc.vector.tensor_add(out=es, in0=ep[:, 0, :], in1=ep[:, 1, :])
                        if jp % 2 == 0:
                            es_prev = es
                        else:
                            es2 = att.tile([P, BLK], F32R, name=f"es2_{ic}_{jp}",
                                           tag="esum", bufs=3)
                            nc.vector.tensor_add(out=es2, in0=es_prev, in1=es)
                            nc.tensor.matmul(psd, ones_r, es2, start=(jp == 1),
                                             stop=(jp == NPR - 1))
                    return pso, psd

                def emit_copies(ic, pso):
                    # free the pso bank ring: 2 copies on DVE, 2 on ScalarE
                    # (all scaled 1/16 for fp8 range; undone via the den recip)
                    osc = att.tile([P, CO, BLK], F8, name=f"osc{ic}", tag="osc",
                                   bufs=2)
                    nc.vector.tensor_scalar_mul(osc[:, 0, :], pso[0], IWS)
                    nc.scalar.activation(out=osc[:, 1, :], in_=pso[1], func=AF.Copy,
                                         scale=IWS)
                    nc.vector.tensor_scalar_mul(osc[:, 2, :], pso[2], IWS)
                    nc.scalar.activation(out=osc[:, 3, :], in_=pso[3], func=AF.Copy,
                                         scale=IWS)
                    return osc

                def emit_tail(ic, psd, osc):
                    den = att.tile([1, BLK], F32R, name=f"den{ic}", tag="den", bufs=2)
                    with nc.allow_low_precision(reason="1/den rounded to f32r for "
                                                "the broadcast matmul; ~1e-4 is "
                                                "within kernel tolerance"):
                        nc.vector.reciprocal(out=den, in_=psd)
                    # broadcast 16/den to all partitions with a K=1 matmul.
                    # net scale: osc carries 1/16, wo carries x16 -> psy is
                    # exact, so rbc needs only the plain 1/den... but osc's
                    # 1/16 and wo's x16 cancel already; keep rbc = 1/den.
                    rbc_ps = ps2.tile([P, BLK], F32, name=f"rbcp{ic}", tag="psd",
                                      bufs=2, space="PSUM")
                    nc.tensor.matmul(rbc_ps, onesrow_r, den, start=True, stop=True)
                    rbc = att.tile([P, BLK], F32, name=f"rbc{ic}", tag="rbc", bufs=2)
                    nc.vector.tensor_copy(out=rbc, in_=rbc_ps)
                    for dt_ in range(CO):
                        psy = ps2.tile([P, BLK], F32, name=f"psy{ic}_{dt_}", tag="pss",
                                       bufs=2, space="PSUM")
                        for t in range(2):
                            nc.tensor.matmul(
                                psy, wo_sb[:, 2 * t:2 * t + 2, dt_ * P:(dt_ + 1) * P],
                                osc[:, 2 * t:2 * t + 2, :], start=(t == 0),
                                stop=(t == 1), perf_mode=DR)
                        y = att.tile([P, BLK], F32, name=f"y{ic}_{dt_}", tag="y",
                                     bufs=2)
                        nc.vector.tensor_mul(out=y, in0=psy, in1=rbc)
                        nc.vector.tensor_scalar_add(y, y, vecs_sb[:, 8 + dt_:9 + dt_])
                        nc.vector.tensor_add(out=y, in0=y,
                                             in1=x_sb[:, dt_, ic * BLK:(ic + 1) * BLK])
                        nc.sync.dma_start(out=out_r[:, dt_, ic * BLK:(ic + 1) * BLK],
                                          in_=y)

                prev = None
                for ic in range(NIC):
                    if prev is not None:
                        osc_p = emit_copies(prev[0], prev[1])
                    cur = (ic, *emit_jloop(ic))
                    if prev is not None:
                        emit_tail(prev[0], prev[2], osc_p)
                    prev = cur
                osc_p = emit_copies(prev[0], prev[1])
                emit_tail(prev[0], prev[2], osc_p)

    nc.compile()
    return nc


def _to_f8(a):
    return np.ascontiguousarray(np.asarray(a, np.float32)).astype(F8NP)


def _make_in_maps(inputs):
    x = np.asarray(inputs["x"], np.float32).reshape(B, C, HW)
    rep = {
        "wq": _to_f8(np.asarray(inputs["wq"], np.float32) * WS),
        "wk": _to_f8(np.asarray(inputs["wk"], np.float32) * WS),
        "wv": _to_f8(inputs["wv"]),
        "wo": _to_f8(np.asarray(inputs["wo"], np.float32) * WS),
        "bq": np.asarray(inputs["bq"], np.float32),
        "bk": np.asarray(inputs["bk"], np.float32),
        "bv": np.asarray(inputs["bv"], np.float32),
        "bo": np.asarray(inputs["bo"], np.float32),
        "gsc": np.asarray(inputs["gn_scale"], np.float32),
        "gbi": np.asarray(inputs["gn_bias"], np.float32),
    }
    emat = np.zeros((C, G), np.float32)
    emat[np.arange(C), np.arange(C) // GSZ] = 1.0
    # emat2[p, t*G+g] = emat[t*P+p, g]; etmat[g, t*P+c] = emat[t*P+c, g]
    rep["emat2"] = np.ascontiguousarray(
        emat.reshape(CO, P, G).transpose(1, 0, 2).reshape(P, CO * G))
    rep["etmat"] = np.ascontiguousarray(emat.T)
    vecs = np.zeros((P, 20), np.float32)
    for i, nm in enumerate(("bq", "bk", "bo", "gsc", "gbi")):
        vecs[:, 4 * i:4 * i + 4] = rep[nm].reshape(CO, P).T
    rep["vecs"] = vecs
    rep["bvbc"] = np.ascontiguousarray(np.broadcast_to(rep["bv"], (P, C)))
    for nm in ("bq", "bk", "bo", "gsc", "gbi", "bv"):
        del rep[nm]
    in_maps = []
    for core in range(8):
        b, half = core // 2, core % 2
        xb = x[b]
        own = xb[:, half * OWN:(half + 1) * OWN]
        oth = xb[:, (1 - half) * OWN:(2 - half) * OWN]
        xp = np.ascontiguousarray(np.concatenate([own, oth], axis=1))
        in_maps.append({"xin": xp, **rep})
    return in_maps


def kernel(**inputs):
    global _CACHED_NC, _LAST
    from concourse.bass_utils import run_bass_kernel_spmd

    if _CACHED_NC is None:
        _CACHED_NC = _build()
    in_maps = _make_in_maps(inputs)
    res = run_bass_kernel_spmd(_CACHED_NC, in_maps, core_ids=list(range(8)))
    _LAST = res
    out = np.empty((B, C, HW), np.float32)
    for core in range(8):
        b, half = core // 2, core % 2
        out[b][:, half * OWN:(half + 1) * OWN] = res.results[core]["out"]
    return out.reshape(B, C, H, W)


# revision 11
# speedup vs baseline: 1.5180x; 1.1909x over previous
"""Trainium2 Bass kernel for nn_AttnBlock_61684320305872.

Computes: GroupNorm(32 groups) -> q/k/v 1x1 convs -> full self-attention over
64x64=4096 spatial positions -> output 1x1 conv -> residual add.

Sharding (8 cores): data-parallel over (batch, spatial-half). Core c handles
batch b=c//2 and query-half h=c%2. Each core computes GroupNorm + full K/V for
its batch (K/V work duplicated across the pair of cores sharing a batch) and
Q + attention rows + projection + residual for its own 2048 positions.
The host permutes each core's spatial axis so its own positions come first;
attention is invariant to key/value ordering, so no unpermute is needed on
the K/V side.

On-chip layout avoids all transposes:
  scores are built transposed  sT[j,i] = sum_d k[d,j] q[d,i]  (lhsT = k slice)
  vT[hw,c] is produced directly by the V projection (lhsT = h_ slice)
  attention out oT[c,i] = sum_j vT[j,c]^T exp_sT[j,i]  accumulates over j
  softmax denominators via a ones-column matmul; 1/den is applied after the
  output projection (it commutes: proj contracts c, den scales per-i).

All heavy matmuls run in fp8e4 with DoubleRow perf mode (2 fp8 weights per PE
cell -> 256-deep contraction per pass, 2x the bf16/f32r MAC rate). Error
analysis: q/k/h/w quantization puts ~0.1 absolute error on softmax logits and
~5% relative error on the attention output, which is itself only ~3% of the
residual magnitude -> final max-abs rel err ~1e-3, far inside the 2e-2 gate.
e4m3 range management: wq/wk/wo are pre-scaled x16 on host (their 1/sqrt(C)
magnitude would land in e4m3's subnormal range) and the x16 is divided out of
the PSUM result on the way to SBUF; exp() gets a -ln8 bias (softmax is
shift-invariant) so the largest weight stays ~100x under the e4m3 max; the
attention accumulator is scaled 1/16 before requantization, with the exact
inverse folded into the softmax denominator reciprocal.

x stays resident in SBUF (read from HBM exactly once); q/k/vT live in SBUF in
fp8 (no DRAM spill); weights arrive host-prequantized in fp8.
"""
import sys

sys.path.insert(0, "/opt/trn_rl_repo")

from contextlib import ExitStack

import ml_dtypes
import numpy as np

import concourse.bass as bass
import concourse.tile as tile
from concourse import bacc, mybir

F32 = mybir.dt.float32
F32R = mybir.dt.float32r
F8 = mybir.dt.float8e4
F8NP = ml_dtypes.float8_e4m3
DR = mybir.MatmulPerfMode.DoubleRow
AF = mybir.ActivationFunctionType
OP = mybir.AluOpType

B, C, H, W = 4, 512, 64, 64
HW = H * W            # 4096 spatial positions
OWN = HW // 2         # 2048 query positions per core
P = 128               # partitions
CO = C // P           # 4 channel chunks
BLK = 512             # spatial block width for streamed phases
NBLK = HW // BLK      # 8
NJT = HW // P         # 32 key tiles
NPR = NJT // 2        # 16 key-tile pairs (DoubleRow granularity)
NIC = OWN // BLK      # 4 query chunks
G = 32                # groups
GSZ = C // G          # 16 channels per group
EPS = 1e-6
SCALE = 1.0 / float(np.sqrt(C))
WS = 16.0             # host pre-scale on wq/wk/wo (fp8 subnormal avoidance)
IWS = 1.0 / WS
EXP_BIAS = -float(np.log(8.0))  # softmax shift: keeps exp() ~100x under e4m3 max

_CACHED_NC = None
_LAST = None


def _build():
    nc = bacc.Bacc("TRN2", target_bir_lowering=False, debug=False, num_devices=8)

    xin = nc.dram_tensor("xin", [C, HW], F32, kind="ExternalInput")
    w_d = {n: nc.dram_tensor(n, [C, C], F8, kind="ExternalInput")
           for n in ("wq", "wk", "wv", "wo")}
    # host-prepacked constants (SBUF layouts; avoids tiny-descriptor DMAs)
    vecs_d = nc.dram_tensor("vecs", [P, 20], F32, kind="ExternalInput")
    bvbc_d = nc.dram_tensor("bvbc", [P, C], F32, kind="ExternalInput")
    emat_d = nc.dram_tensor("emat2", [P, CO * G], F32, kind="ExternalInput")
    etmat_d = nc.dram_tensor("etmat", [G, C], F32, kind="ExternalInput")
    outd = nc.dram_tensor("out", [C, OWN], F32, kind="ExternalOutput")

    x_r = xin.ap().rearrange("(co p) s -> p co s", p=P)
    out_r = outd.ap().rearrange("(co p) s -> p co s", p=P)

    with tile.TileContext(nc) as tc:
        with tc.tile_pool(name="big", bufs=1) as big:
            # ---- long-lived state ----
            x_sb = big.tile([P, CO, HW], F32, name="x_sb", tag="x_sb")
            k_sb = big.tile([P, CO, HW], F8, name="k_sb", tag="k_sb")
            vT_sb = big.tile([P, NJT, C], F8, name="vT_sb", tag="vT_sb")
            q_sb = big.tile([P, CO, OWN], F8, name="q_sb", tag="q_sb")
            a_sb = big.tile([P, CO], F32, name="a_sb", tag="a_sb")
            bsh_sb = big.tile([P, CO], F32, name="bsh_sb", tag="bsh_sb")
            ones8 = big.tile([P, 2, P], F8, name="ones8", tag="ones8")
            nc.vector.memset(ones8, 1.0)
            expb_sb = big.tile([P, 1], F32, name="expb_sb", tag="expb_sb")
            nc.vector.memset(expb_sb, EXP_BIAS)

            vecs_sb = big.tile([P, 20], F32, name="vecs_sb", tag="vecs_sb")
            nc.sync.dma_start(out=vecs_sb, in_=vecs_d.ap())
            bq_sb, bk_sb = vecs_sb[:, 0:4], vecs_sb[:, 4:8]
            gs_sb, gb_sb = vecs_sb[:, 12:16], vecs_sb[:, 16:20]

            with ExitStack() as ph:
                # ---- phase A+B resources (released before attention) ----
                strm = ph.enter_context(tc.tile_pool(name="strm", bufs=3))
                ps1 = ph.enter_context(tc.tile_pool(name="ps1", bufs=1, space="PSUM"))

                # ---- phase A: load all of x into SBUF (once) + GroupNorm
                # statistics. x DMA on the two HWDGE queues (the runtime
                # spreads each across all 16 DMA engines; the GpSimd SWDGE
                # path only adds drain overhead).
                dma_engs = (nc.sync, nc.scalar)
                for s in range(NBLK):
                    dma_engs[s % 2].dma_start(
                        out=x_sb[:, :, s * BLK:(s + 1) * BLK],
                        in_=x_r[:, :, s * BLK:(s + 1) * BLK])
                with tc.tile_pool(name="pa", bufs=3) as pa:
                    stats_sb = pa.tile([P, CO, NBLK, 6], F32, name="stats",
                                       tag="stats", bufs=1)
                    for s in range(NBLK):
                        for co in range(CO):
                            nc.vector.bn_stats(
                                out=stats_sb[:, co, s, :],
                                in_=x_sb[:, co, s * BLK:(s + 1) * BLK])
                    E_sb = pa.tile([P, CO, G], F32, name="E_sb", tag="E_sb", bufs=1)
                    Et_sb = pa.tile([P, CO, P], F32, name="Et_sb", tag="Et_sb", bufs=1)
                    eps_sb = pa.tile([P, 1], F32, name="eps_sb", tag="eps_sb", bufs=1)
                    nc.vector.memset(eps_sb, EPS)
                    nc.sync.dma_start(
                        out=E_sb, in_=emat_d.ap().rearrange("p (t g) -> p t g", g=G))
                    nc.sync.dma_start(
                        out=Et_sb[:G, :, :],
                        in_=etmat_d.ap().rearrange("g (t c) -> g t c", c=P))
                    mv = pa.tile([P, CO, 2], F32, name="mv", tag="mv", bufs=1)
                    t2 = pa.tile([P, CO, 2], F32, name="t2", tag="t2", bufs=1)
                    gw = pa.tile([G, 4], F32, name="gw", tag="gw", bufs=1)
                    gsr = pa.tile([G, 2], F32, name="gsr", tag="gsr", bufs=1)
                    mrs = pa.tile([P, CO, 2], F32, name="mrs", tag="mrs", bufs=1)
                    for co in range(CO):
                        nc.vector.bn_aggr(out=mv[:, co, :], in_=stats_sb[:, co, :, :])
                    # t2 = [mean_c, var_c + mean_c^2] per channel
                    nc.vector.tensor_copy(out=t2[:, :, 0], in_=mv[:, :, 0])
                    nc.vector.tensor_mul(out=t2[:, :, 1], in0=mv[:, :, 0], in1=mv[:, :, 0])
                    nc.vector.tensor_add(out=t2[:, :, 1], in0=t2[:, :, 1], in1=mv[:, :, 1])
                    # group sums via indicator matmul -> [32, 2]
                    psg = ps1.tile([G, 2], F32, name="psg", tag="psg", bufs=1,
                                   space="PSUM")
                    for co in range(CO):
                        nc.tensor.matmul(psg, E_sb[:, co, :], t2[:, co, :],
                                         start=(co == 0), stop=(co == CO - 1))
                    # gw: [group mean, E[var+mean^2], var_g, rstd]
                    nc.scalar.activation(out=gw[:, 0:2], in_=psg, func=AF.Copy,
                                         scale=1.0 / GSZ)
                    nc.vector.tensor_mul(out=gw[:, 2:3], in0=gw[:, 0:1], in1=gw[:, 0:1])
                    nc.vector.tensor_tensor(gw[:, 2:3], gw[:, 1:2], gw[:, 2:3],
                                            OP.subtract)
                    nc.scalar.activation(out=gw[:, 3:4], in_=gw[:, 2:3], func=AF.Sqrt,
                                         bias=eps_sb[:G], scale=1.0)
                    nc.vector.reciprocal(out=gw[:, 3:4], in_=gw[:, 3:4])
                    nc.vector.tensor_copy(out=gsr[:, 0:1], in_=gw[:, 0:1])
                    nc.vector.tensor_copy(out=gsr[:, 1:2], in_=gw[:, 3:4])
                    # broadcast group (mean, rstd) back to channels
                    for co in range(CO):
                        psb = ps1.tile([P, 2], F32, name=f"psb{co}", tag="psbc", bufs=1,
                                       space="PSUM")
                        nc.tensor.matmul(psb, Et_sb[:G, co, :], gsr, start=True,
                                         stop=True)
                        nc.vector.tensor_copy(out=mrs[:, co, :], in_=psb)
                    # h = a*x + b with a = gn_scale*rstd, b = gn_bias - a*mean
                    nc.vector.tensor_mul(out=a_sb, in0=gs_sb, in1=mrs[:, :, 1])
                    nc.vector.tensor_mul(out=bsh_sb, in0=a_sb, in1=mrs[:, :, 0])
                    nc.vector.tensor_tensor(bsh_sb, gb_sb, bsh_sb, OP.subtract)

                # ---- phase B: all projections (fp8 DoubleRow), own-half
                # blocks first so q is ready long before attention reads it.
                bv_bc = strm.tile([P, C], F32, name="bv_bc", tag="bv_bc", bufs=1)
                nc.sync.dma_start(out=bv_bc, in_=bvbc_d.ap())
                pw = ph.enter_context(tc.tile_pool(name="pw", bufs=1))
                wq_sb = pw.tile([P, CO, C], F8, name="wq_sb", tag="wq")
                wk_sb = pw.tile([P, CO, C], F8, name="wk_sb", tag="wk")
                wv_sb = pw.tile([P, CO, C], F8, name="wv_sb", tag="wv")
                for nm, dst, eng in (("wv", wv_sb, nc.sync),
                                     ("wq", wq_sb, nc.scalar),
                                     ("wk", wk_sb, nc.sync)):
                    eng.dma_start(out=dst,
                                  in_=w_d[nm].ap().rearrange("(eo p) d -> p eo d", p=P))
                for s in range(NBLK):
                    # h quantization on GpSimd (SBUF->SBUF): frees DVE for the
                    # vT and q PSUM conversions, which GpSimd cannot touch.
                    hb = strm.tile([P, CO, BLK], F8, name=f"hb{s}", tag="hblk",
                                   bufs=2)
                    for co in range(CO):
                        nc.gpsimd.tensor_scalar(hb[:, co, :],
                                                x_sb[:, co, s * BLK:(s + 1) * BLK],
                                                a_sb[:, co:co + 1],
                                                bsh_sb[:, co:co + 1],
                                                OP.mult, OP.add)
                    if s < NIC:  # own query half
                        for do in range(CO):
                            psq = ps1.tile([P, BLK], F32, name=f"psq{s}_{do}",
                                           tag="psq", bufs=2, space="PSUM")
                            for t in range(2):
                                nc.tensor.matmul(
                                    psq, wq_sb[:, 2 * t:2 * t + 2, do * P:(do + 1) * P],
                                    hb[:, 2 * t:2 * t + 2, :], start=(t == 0),
                                    stop=(t == 1), perf_mode=DR)
                            nc.vector.tensor_scalar(
                                q_sb[:, do, s * BLK:(s + 1) * BLK], psq,
                                IWS, bq_sb[:, do:do + 1], OP.mult, OP.add)
                    for do in range(CO):
                        psk = ps1.tile([P, BLK], F32, name=f"psk{s}_{do}", tag="psk",
                                       bufs=2, space="PSUM")
                        for t in range(2):
                            nc.tensor.matmul(
                                psk, wk_sb[:, 2 * t:2 * t + 2, do * P:(do + 1) * P],
                                hb[:, 2 * t:2 * t + 2, :], start=(t == 0),
                                stop=(t == 1), perf_mode=DR)
                        nc.scalar.activation(out=k_sb[:, do, s * BLK:(s + 1) * BLK],
                                             in_=psk, func=AF.Identity,
                                             bias=bk_sb[:, do:do + 1], scale=IWS)
                    for it in range(BLK // P):
                        psv = ps1.tile([P, C], F32, name=f"psv{s}_{it}", tag="psv",
                                       bufs=2, space="PSUM")
                        for t in range(2):
                            nc.tensor.matmul(
                                psv, hb[:, 2 * t:2 * t + 2, it * P:(it + 1) * P],
                                wv_sb[:, 2 * t:2 * t + 2, :], start=(t == 0),
                                stop=(t == 1), perf_mode=DR)
                        nc.vector.tensor_add(out=vT_sb[:, s * (BLK // P) + it, :],
                                             in0=psv, in1=bv_bc)

            # ---- phase C: attention + projection, per 512-wide query chunk.
            # Software-pipelined emission: chunk ic's PSUM->SBUF copies are
            # emitted before chunk ic+1's score loop (freeing the pso bank
            # ring early), and its projection/epilogue after it (so the PE
            # never waits on the DVE/ACT epilogue at a chunk boundary).
            with tc.tile_pool(name="att", bufs=1) as att, \
                 tc.tile_pool(name="ps2", bufs=1, space="PSUM") as ps2:

                wo_sb = att.tile([P, CO, C], F8, name="wo_sb", tag="wo_sb")
                nc.scalar.dma_start(out=wo_sb,
                                    in_=w_d["wo"].ap().rearrange("(eo p) d -> p eo d",
                                                                 p=P))

                def emit_jloop(ic):
                    pso = [ps2.tile([P, BLK], F32, name=f"pso{ic}_{ct}", tag="pso",
                                    bufs=CO, space="PSUM") for ct in range(CO)]
                    # denominator accumulates via a DoubleRow ones-matmul:
                    # lhsT of all-ones broadcasts den to every partition for
                    # free, so no K=1 broadcast matmul / 1-lane reciprocal.
                    psd = ps2.tile([P, BLK], F32, name=f"psd{ic}", tag="psd", bufs=2,
                                   space="PSUM")
                    for jp in range(NPR):
                        ep = att.tile([P, 2, BLK], F8, name=f"ep{ic}_{jp}", tag="ep",
                                      bufs=3)
                        for jj in range(2):
                            j = 2 * jp + jj
                            pss = ps2.tile([P, BLK], F32, name=f"pss{ic}_{j}",
                                           tag="pss", bufs=2, space="PSUM")
                            for t in range(2):
                                nc.tensor.matmul(
                                    pss, k_sb[:, 2 * t:2 * t + 2, j * P:(j + 1) * P],
                                    q_sb[:, 2 * t:2 * t + 2,
                                         ic * BLK:(ic + 1) * BLK],
                                    start=(t == 0), stop=(t == 1), perf_mode=DR)
                            nc.scalar.activation(out=ep[:, jj, :], in_=pss,
                                                 func=AF.Exp, bias=expb_sb,
                                                 scale=SCALE)
                        for ct in range(CO):
                            nc.tensor.matmul(
                                pso[ct],
                                vT_sb[:, 2 * jp:2 * jp + 2, ct * P:(ct + 1) * P],
                                ep, start=(jp == 0), stop=(jp == NPR - 1),
                                perf_mode=DR)
                        nc.tensor.matmul(psd, ones8, ep, start=(jp == 0),
                                         stop=(jp == NPR - 1), perf_mode=DR)
                    return pso, psd

                def emit_copies(ic, pso):
                    # free the pso bank ring: 2 copies on DVE, 2 on ScalarE
                    # (all scaled 1/16 for fp8 range; undone by wo's x16).
                    # Also precompute the residual+bias tiles here, off the
                    # tail critical path.
                    osc = att.tile([P, CO, BLK], F8, name=f"osc{ic}", tag="osc",
                                   bufs=2)
                    nc.vector.tensor_scalar_mul(osc[:, 0, :], pso[0], IWS)
                    nc.scalar.activation(out=osc[:, 1, :], in_=pso[1], func=AF.Copy,
                                         scale=IWS)
                    nc.vector.tensor_scalar_mul(osc[:, 2, :], pso[2], IWS)
                    nc.scalar.activation(out=osc[:, 3, :], in_=pso[3], func=AF.Copy,
                                         scale=IWS)
                    ypre = att.tile([P, CO, BLK], F32, name=f"ypre{ic}", tag="ypre",
                                    bufs=2)
                    for dt_ in range(CO):
                        nc.vector.tensor_scalar_add(
                            ypre[:, dt_, :], x_sb[:, dt_, ic * BLK:(ic + 1) * BLK],
                            vecs_sb[:, 8 + dt_:9 + dt_])
                    return osc, ypre

                def emit_tail(ic, psd, osc, ypre):
                    # osc carries 1/16 and wo carries x16 -> psy is exactly
                    # sum(exp' * v) per channel; scale by 1/den' elementwise.
                    rbc = att.tile([P, BLK], F32, name=f"rbc{ic}", tag="rbc", bufs=2)
                    nc.vector.reciprocal(out=rbc, in_=psd)
                    for dt_ in range(CO):
                        psy = ps2.tile([P, BLK], F32, name=f"psy{ic}_{dt_}", tag="pss",
                                       bufs=2, space="PSUM")
                        for t in range(2):
                            nc.tensor.matmul(
                                psy, wo_sb[:, 2 * t:2 * t + 2, dt_ * P:(dt_ + 1) * P],
                                osc[:, 2 * t:2 * t + 2, :], start=(t == 0),
                                stop=(t == 1), perf_mode=DR)
                        y = att.tile([P, BLK], F32, name=f"y{ic}_{dt_}", tag="y",
                                     bufs=2)
                        nc.vector.tensor_mul(out=y, in0=psy, in1=rbc)
                        nc.vector.tensor_add(out=y, in0=y, in1=ypre[:, dt_, :])
                        nc.sync.dma_start(out=out_r[:, dt_, ic * BLK:(ic + 1) * BLK],
                                          in_=y)

                prev = None
                for ic in range(NIC):
                    if prev is not None:
                        osc_p, ypre_p = emit_copies(prev[0], prev[1])
                    cur = (ic, *emit_jloop(ic))
                    if prev is not None:
                        emit_tail(prev[0], prev[2], osc_p, ypre_p)
                    prev = cur
                osc_p, ypre_p = emit_copies(prev[0], prev[1])
                emit_tail(prev[0], prev[2], osc_p, ypre_p)

    nc.compile()
    return nc


def _to_f8(a):
    return np.ascontiguousarray(np.asarray(a, np.float32)).astype(F8NP)


def _make_in_maps(inputs):
    x = np.asarray(inputs["x"], np.float32).reshape(B, C, HW)
    rep = {
        "wq": _to_f8(np.asarray(inputs["wq"], np.float32) * WS),
        "wk": _to_f8(np.asarray(inputs["wk"], np.float32) * WS),
        "wv": _to_f8(inputs["wv"]),
        "wo": _to_f8(np.asarray(inputs["wo"], np.float32) * WS),
        "bq": np.asarray(inputs["bq"], np.float32),
        "bk": np.asarray(inputs["bk"], np.float32),
        "bv": np.asarray(inputs["bv"], np.float32),
        "bo": np.asarray(inputs["bo"], np.float32),
        "gsc": np.asarray(inputs["gn_scale"], np.float32),
        "gbi": np.asarray(inputs["gn_bias"], np.float32),
    }
    emat = np.zeros((C, G), np.float32)
    emat[np.arange(C), np.arange(C) // GSZ] = 1.0
    # emat2[p, t*G+g] = emat[t*P+p, g]; etmat[g, t*P+c] = emat[t*P+c, g]
    rep["emat2"] = np.ascontiguousarray(
        emat.reshape(CO, P, G).transpose(1, 0, 2).reshape(P, CO * G))
    rep["etmat"] = np.ascontiguousarray(emat.T)
    vecs = np.zeros((P, 20), np.float32)
    for i, nm in enumerate(("bq", "bk", "bo", "gsc", "gbi")):
        vecs[:, 4 * i:4 * i + 4] = rep[nm].reshape(CO, P).T
    rep["vecs"] = vecs
    rep["bvbc"] = np.ascontiguousarray(np.broadcast_to(rep["bv"], (P, C)))
    for nm in ("bq", "bk", "bo", "gsc", "gbi", "bv"):
        del rep[nm]
    in_maps = []
    for core in range(8):
        b, half = core // 2, core % 2
        xb = x[b]
        own = xb[:, half * OWN:(half + 1) * OWN]
        oth = xb[:, (1 - half) * OWN:(2 - half) * OWN]
        xp = np.ascontiguousarray(np.concatenate([own, oth], axis=1))
        in_maps.append({"xin": xp, **rep})
    return in_maps


def kernel(**inputs):
    global _CACHED_NC, _LAST
    from concourse.bass_utils import run_bass_kernel_spmd

    if _CACHED_NC is None:
        _CACHED_NC = _build()
    in_maps = _make_in_maps(inputs)
    res = run_bass_kernel_spmd(_CACHED_NC, in_maps, core_ids=list(range(8)))
    _LAST = res
    out = np.empty((B, C, HW), np.float32)
    for core in range(8):
        b, half = core // 2, core % 2
        out[b][:, half * OWN:(half + 1) * OWN] = res.results[core]["out"]
    return out.reshape(B, C, H, W)


# revision 19
# speedup vs baseline: 1.7451x; 1.1496x over previous
"""Trainium2 Bass kernel for nn_AttnBlock_61684320305872.

Computes: GroupNorm(32 groups) -> q/k/v 1x1 convs -> full self-attention over
64x64=4096 spatial positions -> output 1x1 conv -> residual add.

Sharding (8 cores): data-parallel over (batch, spatial-half). Core c handles
batch b=c//2 and query-half h=c%2. Each core computes GroupNorm + full K/V for
its batch (K/V work duplicated across the pair of cores sharing a batch) and
Q + attention rows + projection + residual for its own 2048 positions.
The host permutes each core's spatial axis so its own positions come first;
attention is invariant to key/value ordering, so no unpermute is needed on
the K/V side.

On-chip layout avoids all transposes:
  scores are built transposed  sT[j,i] = sum_d k[d,j] q[d,i]  (lhsT = k slice)
  vT[hw,c] is produced directly by the V projection (lhsT = h_ slice)
  attention out oT[c,i] = sum_j vT[j,c]^T exp_sT[j,i]  accumulates over j
  softmax denominators via a ones-column matmul; 1/den is applied after the
  output projection (it commutes: proj contracts c, den scales per-i).

All heavy matmuls run in fp8e4 with DoubleRow perf mode (2 fp8 weights per PE
cell -> 256-deep contraction per pass, 2x the bf16/f32r MAC rate). Error
analysis: q/k/h/w quantization puts ~0.1 absolute error on softmax logits and
~5% relative error on the attention output, which is itself only ~3% of the
residual magnitude -> final max-abs rel err ~1e-3, far inside the 2e-2 gate.
e4m3 range management: wq/wk/wo are pre-scaled x16 on host (their 1/sqrt(C)
magnitude would land in e4m3's subnormal range) and the x16 is divided out of
the PSUM result on the way to SBUF; exp() gets a -ln8 bias (softmax is
shift-invariant) so the largest weight stays ~100x under the e4m3 max; the
attention accumulator is scaled 1/16 before requantization, with the exact
inverse folded into the softmax denominator reciprocal.

x stays resident in SBUF (read from HBM exactly once); q/k/vT live in SBUF in
fp8 (no DRAM spill); weights arrive host-prequantized in fp8.
"""
import sys

sys.path.insert(0, "/opt/trn_rl_repo")

from contextlib import ExitStack

import ml_dtypes
import numpy as np

import concourse.bass as bass
import concourse.tile as tile
from concourse import bacc, mybir

F32 = mybir.dt.float32
F32R = mybir.dt.float32r
F8 = mybir.dt.float8e4
F8NP = ml_dtypes.float8_e4m3
DR = mybir.MatmulPerfMode.DoubleRow
AF = mybir.ActivationFunctionType
OP = mybir.AluOpType

B, C, H, W = 4, 512, 64, 64
HW = H * W            # 4096 spatial positions
OWN = HW // 2         # 2048 query positions per core
P = 128               # partitions
CO = C // P           # 4 channel chunks
BLK = 512             # spatial block width for streamed phases
NBLK = HW // BLK      # 8
NJT = HW // P         # 32 key tiles
NPR = NJT // 2        # 16 key-tile pairs (DoubleRow granularity)
NIC = OWN // BLK      # 4 query chunks
G = 32                # groups
GSZ = C // G          # 16 channels per group
EPS = 1e-6
SCALE = 1.0 / float(np.sqrt(C))
WS = 16.0             # host pre-scale on wq/wk/wo (fp8 subnormal avoidance)
IWS = 1.0 / WS
EXP_BIAS = -float(np.log(8.0))  # softmax shift: keeps exp() ~100x under e4m3 max

_CACHED_NC = None
_LAST = None


def _build():
    nc = bacc.Bacc("TRN2", target_bir_lowering=False, debug=False, num_devices=8)

    xin = nc.dram_tensor("xin", [C, HW], F32, kind="ExternalInput")
    w_d = {n: nc.dram_tensor(n, [C, C], F8, kind="ExternalInput")
           for n in ("wq", "wk", "wv", "wo")}
    # host-prepacked constants (SBUF layouts; avoids tiny-descriptor DMAs)
    vecs_d = nc.dram_tensor("vecs", [P, 20], F32, kind="ExternalInput")
    bvbc_d = nc.dram_tensor("bvbc", [P, C], F32, kind="ExternalInput")
    emat_d = nc.dram_tensor("emat2", [P, CO * G], F32, kind="ExternalInput")
    etmat_d = nc.dram_tensor("etmat", [G, C], F32, kind="ExternalInput")
    outd = nc.dram_tensor("out", [C, OWN], F32, kind="ExternalOutput")

    x_r = xin.ap().rearrange("(co p) s -> p co s", p=P)
    out_r = outd.ap().rearrange("(co p) s -> p co s", p=P)

    with tile.TileContext(nc) as tc:
        with tc.tile_pool(name="big", bufs=1) as big:
            # ---- long-lived state ----
            x_sb = big.tile([P, CO, HW], F32, name="x_sb", tag="x_sb")
            k_sb = big.tile([P, CO, HW], F8, name="k_sb", tag="k_sb")
            vT_sb = big.tile([P, NJT, C], F8, name="vT_sb", tag="vT_sb")
            q_sb = big.tile([P, CO, OWN], F8, name="q_sb", tag="q_sb")
            a_sb = big.tile([P, CO], F32, name="a_sb", tag="a_sb")
            bsh_sb = big.tile([P, CO], F32, name="bsh_sb", tag="bsh_sb")
            ones8 = big.tile([P, 2, P], F8, name="ones8", tag="ones8")
            nc.vector.memset(ones8, 1.0)
            expb_sb = big.tile([P, 1], F32, name="expb_sb", tag="expb_sb")
            nc.vector.memset(expb_sb, EXP_BIAS)

            vecs_sb = big.tile([P, 20], F32, name="vecs_sb", tag="vecs_sb")
            nc.sync.dma_start(out=vecs_sb, in_=vecs_d.ap())
            bq_sb, bk_sb = vecs_sb[:, 0:4], vecs_sb[:, 4:8]
            gs_sb, gb_sb = vecs_sb[:, 12:16], vecs_sb[:, 16:20]

            with ExitStack() as ph:
                # ---- phase A+B resources (released before attention) ----
                strm = ph.enter_context(tc.tile_pool(name="strm", bufs=3))
                ps1 = ph.enter_context(tc.tile_pool(name="ps1", bufs=1, space="PSUM"))

                # ---- phase A: load all of x into SBUF (once) + GroupNorm
                # statistics. x DMA on the two HWDGE queues (the runtime
                # spreads each across all 16 DMA engines; the GpSimd SWDGE
                # path only adds drain overhead).
                dma_engs = (nc.sync, nc.scalar)
                for s in range(NBLK):
                    dma_engs[s % 2].dma_start(
                        out=x_sb[:, :, s * BLK:(s + 1) * BLK],
                        in_=x_r[:, :, s * BLK:(s + 1) * BLK])
                with tc.tile_pool(name="pa", bufs=3) as pa:
                    stats_sb = pa.tile([P, CO, NBLK, 6], F32, name="stats",
                                       tag="stats", bufs=1)
                    for s in range(NBLK):
                        for co in range(CO):
                            nc.vector.bn_stats(
                                out=stats_sb[:, co, s, :],
                                in_=x_sb[:, co, s * BLK:(s + 1) * BLK])
                    E_sb = pa.tile([P, CO, G], F32, name="E_sb", tag="E_sb", bufs=1)
                    Et_sb = pa.tile([P, CO, P], F32, name="Et_sb", tag="Et_sb", bufs=1)
                    eps_sb = pa.tile([P, 1], F32, name="eps_sb", tag="eps_sb", bufs=1)
                    nc.vector.memset(eps_sb, EPS)
                    nc.sync.dma_start(
                        out=E_sb, in_=emat_d.ap().rearrange("p (t g) -> p t g", g=G))
                    nc.sync.dma_start(
                        out=Et_sb[:G, :, :],
                        in_=etmat_d.ap().rearrange("g (t c) -> g t c", c=P))
                    mv = pa.tile([P, CO, 2], F32, name="mv", tag="mv", bufs=1)
                    t2 = pa.tile([P, CO, 2], F32, name="t2", tag="t2", bufs=1)
                    gw = pa.tile([G, 4], F32, name="gw", tag="gw", bufs=1)
                    gsr = pa.tile([G, 2], F32, name="gsr", tag="gsr", bufs=1)
                    mrs = pa.tile([P, CO, 2], F32, name="mrs", tag="mrs", bufs=1)
                    for co in range(CO):
                        nc.vector.bn_aggr(out=mv[:, co, :], in_=stats_sb[:, co, :, :])
                    # t2 = [mean_c, var_c + mean_c^2] per channel
                    nc.vector.tensor_copy(out=t2[:, :, 0], in_=mv[:, :, 0])
                    nc.vector.tensor_mul(out=t2[:, :, 1], in0=mv[:, :, 0], in1=mv[:, :, 0])
                    nc.vector.tensor_add(out=t2[:, :, 1], in0=t2[:, :, 1], in1=mv[:, :, 1])
                    # group sums via indicator matmul -> [32, 2]
                    psg = ps1.tile([G, 2], F32, name="psg", tag="psg", bufs=1,
                                   space="PSUM")
                    for co in range(CO):
                        nc.tensor.matmul(psg, E_sb[:, co, :], t2[:, co, :],
                                         start=(co == 0), stop=(co == CO - 1))
                    # gw: [group mean, E[var+mean^2], var_g, rstd]
                    nc.scalar.activation(out=gw[:, 0:2], in_=psg, func=AF.Copy,
                                         scale=1.0 / GSZ)
                    nc.vector.tensor_mul(out=gw[:, 2:3], in0=gw[:, 0:1], in1=gw[:, 0:1])
                    nc.vector.tensor_tensor(gw[:, 2:3], gw[:, 1:2], gw[:, 2:3],
                                            OP.subtract)
                    nc.scalar.activation(out=gw[:, 3:4], in_=gw[:, 2:3], func=AF.Sqrt,
                                         bias=eps_sb[:G], scale=1.0)
                    nc.vector.reciprocal(out=gw[:, 3:4], in_=gw[:, 3:4])
                    nc.vector.tensor_copy(out=gsr[:, 0:1], in_=gw[:, 0:1])
                    nc.vector.tensor_copy(out=gsr[:, 1:2], in_=gw[:, 3:4])
                    # broadcast group (mean, rstd) back to channels
                    for co in range(CO):
                        psb = ps1.tile([P, 2], F32, name=f"psb{co}", tag="psbc", bufs=1,
                                       space="PSUM")
                        nc.tensor.matmul(psb, Et_sb[:G, co, :], gsr, start=True,
                                         stop=True)
                        nc.vector.tensor_copy(out=mrs[:, co, :], in_=psb)
                    # h = a*x + b with a = gn_scale*rstd, b = gn_bias - a*mean
                    nc.vector.tensor_mul(out=a_sb, in0=gs_sb, in1=mrs[:, :, 1])
                    nc.vector.tensor_mul(out=bsh_sb, in0=a_sb, in1=mrs[:, :, 0])
                    nc.vector.tensor_tensor(bsh_sb, gb_sb, bsh_sb, OP.subtract)

                # ---- phase B: all projections (fp8 DoubleRow), own-half
                # blocks first so q is ready long before attention reads it.
                bv_bc = strm.tile([P, C], F32, name="bv_bc", tag="bv_bc", bufs=1)
                nc.sync.dma_start(out=bv_bc, in_=bvbc_d.ap())
                pw = ph.enter_context(tc.tile_pool(name="pw", bufs=1))
                wq_sb = pw.tile([P, CO, C], F8, name="wq_sb", tag="wq")
                wk_sb = pw.tile([P, CO, C], F8, name="wk_sb", tag="wk")
                wv_sb = pw.tile([P, CO, C], F8, name="wv_sb", tag="wv")
                for nm, dst, eng in (("wv", wv_sb, nc.sync),
                                     ("wq", wq_sb, nc.scalar),
                                     ("wk", wk_sb, nc.sync)):
                    eng.dma_start(out=dst,
                                  in_=w_d[nm].ap().rearrange("(eo p) d -> p eo d", p=P))
                for s in range(NBLK):
                    # h quantization on GpSimd (SBUF->SBUF): frees DVE for the
                    # vT and q PSUM conversions, which GpSimd cannot touch.
                    hb = strm.tile([P, CO, BLK], F8, name=f"hb{s}", tag="hblk",
                                   bufs=2)
                    for co in range(CO):
                        # first block split across DVE+GpSimd: it gates the
                        # first projection matmul right after the GN stats
                        eng = nc.vector if (s == 0 and co < 2) else nc.gpsimd
                        eng.tensor_scalar(hb[:, co, :],
                                          x_sb[:, co, s * BLK:(s + 1) * BLK],
                                          a_sb[:, co:co + 1],
                                          bsh_sb[:, co:co + 1],
                                          OP.mult, OP.add)
                    if s < NIC:  # own query half
                        for do in range(CO):
                            psq = ps1.tile([P, BLK], F32, name=f"psq{s}_{do}",
                                           tag="psq", bufs=2, space="PSUM")
                            for t in range(2):
                                nc.tensor.matmul(
                                    psq, wq_sb[:, 2 * t:2 * t + 2, do * P:(do + 1) * P],
                                    hb[:, 2 * t:2 * t + 2, :], start=(t == 0),
                                    stop=(t == 1), perf_mode=DR)
                            nc.vector.tensor_scalar(
                                q_sb[:, do, s * BLK:(s + 1) * BLK], psq,
                                IWS, bq_sb[:, do:do + 1], OP.mult, OP.add)
                    for do in range(CO):
                        psk = ps1.tile([P, BLK], F32, name=f"psk{s}_{do}", tag="psk",
                                       bufs=2, space="PSUM")
                        for t in range(2):
                            nc.tensor.matmul(
                                psk, wk_sb[:, 2 * t:2 * t + 2, do * P:(do + 1) * P],
                                hb[:, 2 * t:2 * t + 2, :], start=(t == 0),
                                stop=(t == 1), perf_mode=DR)
                        nc.scalar.activation(out=k_sb[:, do, s * BLK:(s + 1) * BLK],
                                             in_=psk, func=AF.Identity,
                                             bias=bk_sb[:, do:do + 1], scale=IWS)
                    for it in range(BLK // P):
                        psv = ps1.tile([P, C], F32, name=f"psv{s}_{it}", tag="psv",
                                       bufs=2, space="PSUM")
                        for t in range(2):
                            nc.tensor.matmul(
                                psv, hb[:, 2 * t:2 * t + 2, it * P:(it + 1) * P],
                                wv_sb[:, 2 * t:2 * t + 2, :], start=(t == 0),
                                stop=(t == 1), perf_mode=DR)
                        nc.vector.tensor_add(out=vT_sb[:, s * (BLK // P) + it, :],
                                             in0=psv, in1=bv_bc)

            # ---- phase C: attention + projection, per 512-wide query chunk.
            # Software-pipelined emission: chunk ic's PSUM->SBUF copies are
            # emitted before chunk ic+1's score loop (freeing the pso bank
            # ring early), and its projection/epilogue after it (so the PE
            # never waits on the DVE/ACT epilogue at a chunk boundary).
            with tc.tile_pool(name="att", bufs=1) as att, \
                 tc.tile_pool(name="ps2", bufs=1, space="PSUM") as ps2:

                wo_sb = att.tile([P, CO, C], F8, name="wo_sb", tag="wo_sb")
                nc.scalar.dma_start(out=wo_sb,
                                    in_=w_d["wo"].ap().rearrange("(eo p) d -> p eo d",
                                                                 p=P))

                def emit_jloop(ic):
                    pso = [ps2.tile([P, BLK], F32, name=f"pso{ic}_{ct}", tag="pso",
                                    bufs=CO, space="PSUM") for ct in range(CO)]
                    # denominator accumulates via a DoubleRow ones-matmul:
                    # lhsT of all-ones broadcasts den to every partition for
                    # free, so no K=1 broadcast matmul / 1-lane reciprocal.
                    psd = ps2.tile([P, BLK], F32, name=f"psd{ic}", tag="psd", bufs=1,
                                   space="PSUM")
                    for jp in range(NPR):
                        ep = att.tile([P, 2, BLK], F8, name=f"ep{ic}_{jp}", tag="ep",
                                      bufs=4)
                        for jj in range(2):
                            j = 2 * jp + jj
                            pss = ps2.tile([P, BLK], F32, name=f"pss{ic}_{j}",
                                           tag="pss", bufs=3, space="PSUM")
                            for t in range(2):
                                nc.tensor.matmul(
                                    pss, k_sb[:, 2 * t:2 * t + 2, j * P:(j + 1) * P],
                                    q_sb[:, 2 * t:2 * t + 2,
                                         ic * BLK:(ic + 1) * BLK],
                                    start=(t == 0), stop=(t == 1), perf_mode=DR)
                            nc.scalar.activation(out=ep[:, jj, :], in_=pss,
                                                 func=AF.Exp, bias=expb_sb,
                                                 scale=SCALE)
                        for ct in range(CO):
                            nc.tensor.matmul(
                                pso[ct],
                                vT_sb[:, 2 * jp:2 * jp + 2, ct * P:(ct + 1) * P],
                                ep, start=(jp == 0), stop=(jp == NPR - 1),
                                perf_mode=DR)
                        nc.tensor.matmul(psd, ones8, ep, start=(jp == 0),
                                         stop=(jp == NPR - 1), perf_mode=DR)
                    return pso, psd

                def emit_copies(ic, pso, psd):
                    # The ~3.4us reciprocal runs FIRST on DVE, during the next
                    # chunk's score loop (DVE is idle there) — psd finished
                    # with this chunk's j-loop, and psd's single bank is only
                    # recycled once the reciprocal has read it.
                    rbc = att.tile([P, BLK], F32, name=f"rbc{ic}", tag="rbc", bufs=2)
                    nc.vector.reciprocal(out=rbc, in_=psd)
                    # free the pso bank ring: 2 copies on DVE, 2 on ScalarE
                    # (all scaled 1/16 for fp8 range; undone by wo's x16).
                    # Also precompute the residual+bias tiles here, off the
                    # tail critical path.
                    osc = att.tile([P, CO, BLK], F8, name=f"osc{ic}", tag="osc",
                                   bufs=2)
                    nc.vector.tensor_scalar_mul(osc[:, 0, :], pso[0], IWS)
                    nc.scalar.activation(out=osc[:, 1, :], in_=pso[1], func=AF.Copy,
                                         scale=IWS)
                    nc.vector.tensor_scalar_mul(osc[:, 2, :], pso[2], IWS)
                    nc.scalar.activation(out=osc[:, 3, :], in_=pso[3], func=AF.Copy,
                                         scale=IWS)
                    ypre = att.tile([P, CO, BLK], F32, name=f"ypre{ic}", tag="ypre",
                                    bufs=2)
                    for dt_ in range(CO):
                        nc.vector.tensor_scalar_add(
                            ypre[:, dt_, :], x_sb[:, dt_, ic * BLK:(ic + 1) * BLK],
                            vecs_sb[:, 8 + dt_:9 + dt_])
                    return osc, ypre, rbc

                def emit_tail(ic, osc, ypre, rbc):
                    # osc carries 1/16 and wo carries x16 -> psy is exactly
                    # sum(exp' * v) per channel; scale by 1/den' elementwise.
                    for dt_ in range(CO):
                        psy = ps2.tile([P, BLK], F32, name=f"psy{ic}_{dt_}", tag="pss",
                                       bufs=3, space="PSUM")
                        for t in range(2):
                            nc.tensor.matmul(
                                psy, wo_sb[:, 2 * t:2 * t + 2, dt_ * P:(dt_ + 1) * P],
                                osc[:, 2 * t:2 * t + 2, :], start=(t == 0),
                                stop=(t == 1), perf_mode=DR)
                        y = att.tile([P, BLK], F32, name=f"y{ic}_{dt_}", tag="y",
                                     bufs=2)
                        nc.vector.tensor_mul(out=y, in0=psy, in1=rbc)
                        nc.gpsimd.tensor_add(out=y, in0=y, in1=ypre[:, dt_, :])
                        nc.sync.dma_start(out=out_r[:, dt_, ic * BLK:(ic + 1) * BLK],
                                          in_=y)

                prev = None
                for ic in range(NIC):
                    if prev is not None:
                        tail_args = emit_copies(prev[0], prev[1], prev[2])
                    cur = (ic, *emit_jloop(ic))
                    if prev is not None:
                        emit_tail(prev[0], *tail_args)
                    prev = cur
                tail_args = emit_copies(prev[0], prev[1], prev[2])
                emit_tail(prev[0], *tail_args)

    nc.compile()
    return nc


def _to_f8(a):
    return np.ascontiguousarray(np.asarray(a, np.float32)).astype(F8NP)


def _make_in_maps(inputs):
    x = np.asarray(inputs["x"], np.float32).reshape(B, C, HW)
    rep = {
        "wq": _to_f8(np.asarray(inputs["wq"], np.float32) * WS),
        "wk": _to_f8(np.asarray(inputs["wk"], np.float32) * WS),
        "wv": _to_f8(inputs["wv"]),
        "wo": _to_f8(np.asarray(inputs["wo"], np.float32) * WS),
        "bq": np.asarray(inputs["bq"], np.float32),
        "bk": np.asarray(inputs["bk"], np.float32),
        "bv": np.asarray(inputs["bv"], np.float32),
        "bo": np.asarray(inputs["bo"], np.float32),
        "gsc": np.asarray(inputs["gn_scale"], np.float32),
        "gbi": np.asarray(inputs["gn_bias"], np.float32),
    }
    emat = np.zeros((C, G), np.float32)
    emat[np.arange(C), np.arange(C) // GSZ] = 1.0
    # emat2[p, t*G+g] = emat[t*P+p, g]; etmat[g, t*P+c] = emat[t*P+c, g]
    rep["emat2"] = np.ascontiguousarray(
        emat.reshape(CO, P, G).transpose(1, 0, 2).reshape(P, CO * G))
    rep["etmat"] = np.ascontiguousarray(emat.T)
    vecs = np.zeros((P, 20), np.float32)
    for i, nm in enumerate(("bq", "bk", "bo", "gsc", "gbi")):
        vecs[:, 4 * i:4 * i + 4] = rep[nm].reshape(CO, P).T
    rep["vecs"] = vecs
    rep["bvbc"] = np.ascontiguousarray(np.broadcast_to(rep["bv"], (P, C)))
    for nm in ("bq", "bk", "bo", "gsc", "gbi", "bv"):
        del rep[nm]
    in_maps = []
    for core in range(8):
        b, half = core // 2, core % 2
        xb = x[b]
        own = xb[:, half * OWN:(half + 1) * OWN]
        oth = xb[:, (1 - half) * OWN:(2 - half) * OWN]
        xp = np.ascontiguousarray(np.concatenate([own, oth], axis=1))
        in_maps.append({"xin": xp, **rep})
    return in_maps


def kernel(**inputs):
    global _CACHED_NC, _LAST
    from concourse.bass_utils import run_bass_kernel_spmd

    if _CACHED_NC is None:
        _CACHED_NC = _build()
    in_maps = _make_in_maps(inputs)
    res = run_bass_kernel_spmd(_CACHED_NC, in_maps, core_ids=list(range(8)))
    _LAST = res
    out = np.empty((B, C, HW), np.float32)
    for core in range(8):
        b, half = core // 2, core % 2
        out[b][:, half * OWN:(half + 1) * OWN] = res.results[core]["out"]
    return out.reshape(B, C, H, W)


# revision 30
# speedup vs baseline: 1.7804x; 1.0202x over previous
"""Trainium2 Bass kernel for nn_AttnBlock_61684320305872.

Computes: GroupNorm(32 groups) -> q/k/v 1x1 convs -> full self-attention over
64x64=4096 spatial positions -> output 1x1 conv -> residual add.

Sharding (8 cores): data-parallel over (batch, spatial-half). Core c handles
batch b=c//2 and query-half h=c%2. Each core computes GroupNorm + full K/V for
its batch (K/V work duplicated across the pair of cores sharing a batch) and
Q + attention rows + projection + residual for its own 2048 positions.
The host permutes each core's spatial axis so its own positions come first;
attention is invariant to key/value ordering, so no unpermute is needed on
the K/V side.

On-chip layout avoids all transposes:
  scores are built transposed  sT[j,i] = sum_d k[d,j] q[d,i]  (lhsT = k slice)
  vT[hw,c] is produced directly by the V projection (lhsT = h_ slice)
  attention out oT[c,i] = sum_j vT[j,c]^T exp_sT[j,i]  accumulates over j
  softmax denominators via a ones-column matmul; 1/den is applied after the
  output projection (it commutes: proj contracts c, den scales per-i).

All heavy matmuls run in fp8e4 with DoubleRow perf mode (2 fp8 weights per PE
cell -> 256-deep contraction per pass, 2x the bf16/f32r MAC rate). Error
analysis: q/k/h/w quantization puts ~0.1 absolute error on softmax logits and
~5% relative error on the attention output, which is itself only ~3% of the
residual magnitude -> final max-abs rel err ~1e-3, far inside the 2e-2 gate.
e4m3 range management: wq/wk/wo are pre-scaled x16 on host (their 1/sqrt(C)
magnitude would land in e4m3's subnormal range) and the x16 is divided out of
the PSUM result on the way to SBUF; exp() gets a -ln8 bias (softmax is
shift-invariant) so the largest weight stays ~100x under the e4m3 max; the
attention accumulator is scaled 1/16 before requantization, with the exact
inverse folded into the softmax denominator reciprocal.

x stays resident in SBUF (read from HBM exactly once); q/k/vT live in SBUF in
fp8 (no DRAM spill); weights arrive host-prequantized in fp8.
"""
import sys

sys.path.insert(0, "/opt/trn_rl_repo")

from contextlib import ExitStack

import ml_dtypes
import numpy as np

import concourse.bass as bass
import concourse.tile as tile
from concourse import bacc, mybir

F32 = mybir.dt.float32
F32R = mybir.dt.float32r
F8 = mybir.dt.float8e4
F8NP = ml_dtypes.float8_e4m3
DR = mybir.MatmulPerfMode.DoubleRow
AF = mybir.ActivationFunctionType
OP = mybir.AluOpType

B, C, H, W = 4, 512, 64, 64
HW = H * W            # 4096 spatial positions
OWN = HW // 2         # 2048 query positions per core
P = 128               # partitions
CO = C // P           # 4 channel chunks
BLK = 512             # spatial block width for streamed phases
NBLK = HW // BLK      # 8
NJT = HW // P         # 32 key tiles
NPR = NJT // 2        # 16 key-tile pairs (DoubleRow granularity)
NIC = OWN // BLK      # 4 query chunks
G = 32                # groups
GSZ = C // G          # 16 channels per group
EPS = 1e-6
SCALE = 1.0 / float(np.sqrt(C))
WS = 16.0             # host pre-scale on wq/wk/wo (fp8 subnormal avoidance)
IWS = 1.0 / WS
EXP_BIAS = -float(np.log(8.0))  # softmax shift: keeps exp() ~100x under e4m3 max

_CACHED_NC = None
_LAST = None


def _build():
    nc = bacc.Bacc("TRN2", target_bir_lowering=False, debug=False, num_devices=8)

    xin = nc.dram_tensor("xin", [C, HW], F32, kind="ExternalInput")
    w_d = {n: nc.dram_tensor(n, [C, C], F8, kind="ExternalInput")
           for n in ("wq", "wk", "wv", "wo")}
    # host-prepacked constants (SBUF layouts; avoids tiny-descriptor DMAs)
    vecs_d = nc.dram_tensor("vecs", [P, 20], F32, kind="ExternalInput")
    bvbc_d = nc.dram_tensor("bvbc", [P, C], F32, kind="ExternalInput")
    emat_d = nc.dram_tensor("emat2", [P, CO * G], F32, kind="ExternalInput")
    etmat_d = nc.dram_tensor("etmat", [G, C], F32, kind="ExternalInput")
    outd = nc.dram_tensor("out", [C, OWN], F32, kind="ExternalOutput")

    x_r = xin.ap().rearrange("(co p) s -> p co s", p=P)
    out_r = outd.ap().rearrange("(co p) s -> p co s", p=P)

    with tile.TileContext(nc) as tc:
        with tc.tile_pool(name="big", bufs=1) as big:
            # ---- long-lived state ----
            x_sb = big.tile([P, CO, HW], F32, name="x_sb", tag="x_sb")
            k_sb = big.tile([P, CO, HW], F8, name="k_sb", tag="k_sb")
            vT_sb = big.tile([P, NJT, C], F8, name="vT_sb", tag="vT_sb")
            q_sb = big.tile([P, CO, OWN], F8, name="q_sb", tag="q_sb")
            a_sb = big.tile([P, CO], F32, name="a_sb", tag="a_sb")
            bsh_sb = big.tile([P, CO], F32, name="bsh_sb", tag="bsh_sb")
            ones8 = big.tile([P, 2, P], F8, name="ones8", tag="ones8")
            nc.vector.memset(ones8, 1.0)
            expb_sb = big.tile([P, 1], F32, name="expb_sb", tag="expb_sb")
            nc.vector.memset(expb_sb, EXP_BIAS)

            vecs_sb = big.tile([P, 20], F32, name="vecs_sb", tag="vecs_sb")
            nc.sync.dma_start(out=vecs_sb, in_=vecs_d.ap())
            bq_sb, bk_sb = vecs_sb[:, 0:4], vecs_sb[:, 4:8]
            gs_sb, gb_sb = vecs_sb[:, 12:16], vecs_sb[:, 16:20]

            with ExitStack() as ph:
                # ---- phase A+B resources (released before attention) ----
                strm = ph.enter_context(tc.tile_pool(name="strm", bufs=3))
                ps1 = ph.enter_context(tc.tile_pool(name="ps1", bufs=1, space="PSUM"))

                # ---- phase A: load all of x into SBUF (once) + GroupNorm
                # statistics. x DMA on the two HWDGE queues (the runtime
                # spreads each across all 16 DMA engines; the GpSimd SWDGE
                # path only adds drain overhead).
                dma_engs = (nc.sync, nc.scalar)
                for s in range(NBLK):
                    dma_engs[s % 2].dma_start(
                        out=x_sb[:, :, s * BLK:(s + 1) * BLK],
                        in_=x_r[:, :, s * BLK:(s + 1) * BLK])
                with tc.tile_pool(name="pa", bufs=3) as pa:
                    stats_sb = pa.tile([P, CO, NBLK, 6], F32, name="stats",
                                       tag="stats", bufs=1)
                    for s in range(NBLK):
                        for co in range(CO):
                            nc.vector.bn_stats(
                                out=stats_sb[:, co, s, :],
                                in_=x_sb[:, co, s * BLK:(s + 1) * BLK])
                    E_sb = pa.tile([P, CO, G], F32, name="E_sb", tag="E_sb", bufs=1)
                    Et_sb = pa.tile([P, CO, P], F32, name="Et_sb", tag="Et_sb", bufs=1)
                    eps_sb = pa.tile([P, 1], F32, name="eps_sb", tag="eps_sb", bufs=1)
                    nc.vector.memset(eps_sb, EPS)
                    nc.sync.dma_start(
                        out=E_sb, in_=emat_d.ap().rearrange("p (t g) -> p t g", g=G))
                    nc.sync.dma_start(
                        out=Et_sb[:G, :, :],
                        in_=etmat_d.ap().rearrange("g (t c) -> g t c", c=P))
                    mv = pa.tile([P, CO, 2], F32, name="mv", tag="mv", bufs=1)
                    t2 = pa.tile([P, CO, 2], F32, name="t2", tag="t2", bufs=1)
                    gw = pa.tile([G, 4], F32, name="gw", tag="gw", bufs=1)
                    gsr = pa.tile([G, 2], F32, name="gsr", tag="gsr", bufs=1)
                    mrs = pa.tile([P, CO, 2], F32, name="mrs", tag="mrs", bufs=1)
                    for co in range(CO):
                        nc.vector.bn_aggr(out=mv[:, co, :], in_=stats_sb[:, co, :, :])
                    # t2 = [mean_c, var_c + mean_c^2] per channel
                    nc.vector.tensor_copy(out=t2[:, :, 0], in_=mv[:, :, 0])
                    nc.vector.tensor_mul(out=t2[:, :, 1], in0=mv[:, :, 0], in1=mv[:, :, 0])
                    nc.vector.tensor_add(out=t2[:, :, 1], in0=t2[:, :, 1], in1=mv[:, :, 1])
                    # group sums via indicator matmul -> [32, 2]
                    # GN matmul outputs share the phase-B psv bank ring (the
                    # rings are sequential in time; a second PSUM pool cannot
                    # coexist with ps1)
                    psg = ps1.tile([G, 2], F32, name="psg", tag="psv", bufs=3,
                                   space="PSUM")
                    for co in range(CO):
                        nc.tensor.matmul(psg, E_sb[:, co, :], t2[:, co, :],
                                         start=(co == 0), stop=(co == CO - 1))
                    # gw: [group mean, E[var+mean^2], var_g, rstd]
                    nc.scalar.activation(out=gw[:, 0:2], in_=psg, func=AF.Copy,
                                         scale=1.0 / GSZ)
                    nc.vector.tensor_mul(out=gw[:, 2:3], in0=gw[:, 0:1], in1=gw[:, 0:1])
                    nc.vector.tensor_tensor(gw[:, 2:3], gw[:, 1:2], gw[:, 2:3],
                                            OP.subtract)
                    nc.scalar.activation(out=gw[:, 3:4], in_=gw[:, 2:3], func=AF.Sqrt,
                                         bias=eps_sb[:G], scale=1.0)
                    nc.vector.reciprocal(out=gw[:, 3:4], in_=gw[:, 3:4])
                    nc.vector.tensor_copy(out=gsr[:, 0:1], in_=gw[:, 0:1])
                    nc.vector.tensor_copy(out=gsr[:, 1:2], in_=gw[:, 3:4])
                    # broadcast group (mean, rstd) back to channels
                    for co in range(CO):
                        psb = ps1.tile([P, 2], F32, name=f"psb{co}", tag="psv", bufs=3,
                                       space="PSUM")
                        nc.tensor.matmul(psb, Et_sb[:G, co, :], gsr, start=True,
                                         stop=True)
                        nc.vector.tensor_copy(out=mrs[:, co, :], in_=psb)
                    # h = a*x + b with a = gn_scale*rstd, b = gn_bias - a*mean
                    nc.vector.tensor_mul(out=a_sb, in0=gs_sb, in1=mrs[:, :, 1])
                    nc.vector.tensor_mul(out=bsh_sb, in0=a_sb, in1=mrs[:, :, 0])
                    nc.vector.tensor_tensor(bsh_sb, gb_sb, bsh_sb, OP.subtract)

                # ---- phase B: all projections (fp8 DoubleRow), own-half
                # blocks first so q is ready long before attention reads it.
                bv_bc = strm.tile([P, C], F32, name="bv_bc", tag="bv_bc", bufs=1)
                nc.sync.dma_start(out=bv_bc, in_=bvbc_d.ap())
                pw = ph.enter_context(tc.tile_pool(name="pw", bufs=1))
                wq_sb = pw.tile([P, CO, C], F8, name="wq_sb", tag="wq")
                wk_sb = pw.tile([P, CO, C], F8, name="wk_sb", tag="wk")
                wv_sb = pw.tile([P, CO, C], F8, name="wv_sb", tag="wv")
                for nm, dst, eng in (("wv", wv_sb, nc.sync),
                                     ("wq", wq_sb, nc.scalar),
                                     ("wk", wk_sb, nc.sync)):
                    eng.dma_start(out=dst,
                                  in_=w_d[nm].ap().rearrange("(eo p) d -> p eo d", p=P))
                for s in range(NBLK):
                    # h quantization on GpSimd (SBUF->SBUF): frees DVE for the
                    # vT and q PSUM conversions, which GpSimd cannot touch.
                    hb = strm.tile([P, CO, BLK], F8, name=f"hb{s}", tag="hblk",
                                   bufs=2)
                    for co in range(CO):
                        # first block split across DVE+GpSimd: it gates the
                        # first projection matmul right after the GN stats
                        eng = nc.vector if (s == 0 and co < 2) else nc.gpsimd
                        eng.tensor_scalar(hb[:, co, :],
                                          x_sb[:, co, s * BLK:(s + 1) * BLK],
                                          a_sb[:, co:co + 1],
                                          bsh_sb[:, co:co + 1],
                                          OP.mult, OP.add)
                    if s < NIC:  # own query half
                        for do in range(CO):
                            psq = ps1.tile([P, BLK], F32, name=f"psq{s}_{do}",
                                           tag="psq", bufs=3, space="PSUM")
                            for t in range(2):
                                nc.tensor.matmul(
                                    psq, wq_sb[:, 2 * t:2 * t + 2, do * P:(do + 1) * P],
                                    hb[:, 2 * t:2 * t + 2, :], start=(t == 0),
                                    stop=(t == 1), perf_mode=DR)
                            # q conversion split DVE/ScalarE (DVE alone would
                            # gate the block loop; ScalarE has headroom)
                            if do < 2:
                                nc.scalar.activation(
                                    out=q_sb[:, do, s * BLK:(s + 1) * BLK], in_=psq,
                                    func=AF.Identity, bias=bq_sb[:, do:do + 1],
                                    scale=IWS)
                            else:
                                nc.vector.tensor_scalar(
                                    q_sb[:, do, s * BLK:(s + 1) * BLK], psq,
                                    IWS, bq_sb[:, do:do + 1], OP.mult, OP.add)
                    for do in range(CO):
                        psk = ps1.tile([P, BLK], F32, name=f"psk{s}_{do}", tag="psk",
                                       bufs=2, space="PSUM")
                        for t in range(2):
                            nc.tensor.matmul(
                                psk, wk_sb[:, 2 * t:2 * t + 2, do * P:(do + 1) * P],
                                hb[:, 2 * t:2 * t + 2, :], start=(t == 0),
                                stop=(t == 1), perf_mode=DR)
                        nc.scalar.activation(out=k_sb[:, do, s * BLK:(s + 1) * BLK],
                                             in_=psk, func=AF.Identity,
                                             bias=bk_sb[:, do:do + 1], scale=IWS)
                    for it in range(BLK // P):
                        psv = ps1.tile([P, C], F32, name=f"psv{s}_{it}", tag="psv",
                                       bufs=3, space="PSUM")
                        for t in range(2):
                            nc.tensor.matmul(
                                psv, hb[:, 2 * t:2 * t + 2, it * P:(it + 1) * P],
                                wv_sb[:, 2 * t:2 * t + 2, :], start=(t == 0),
                                stop=(t == 1), perf_mode=DR)
                        nc.vector.tensor_add(out=vT_sb[:, s * (BLK // P) + it, :],
                                             in0=psv, in1=bv_bc)

            # ---- phase C: attention + projection, per 512-wide query chunk.
            # Software-pipelined emission: chunk ic's PSUM->SBUF copies are
            # emitted before chunk ic+1's score loop (freeing the pso bank
            # ring early), and its projection/epilogue after it (so the PE
            # never waits on the DVE/ACT epilogue at a chunk boundary).
            with tc.tile_pool(name="att", bufs=1) as att, \
                 tc.tile_pool(name="ps2", bufs=1, space="PSUM") as ps2:

                wo_sb = att.tile([P, CO, C], F8, name="wo_sb", tag="wo_sb")
                nc.scalar.dma_start(out=wo_sb,
                                    in_=w_d["wo"].ap().rearrange("(eo p) d -> p eo d",
                                                                 p=P))

                def emit_jloop(ic):
                    pso = [ps2.tile([P, BLK], F32, name=f"pso{ic}_{ct}", tag="pso",
                                    bufs=CO, space="PSUM") for ct in range(CO)]
                    # denominator accumulates via a DoubleRow ones-matmul:
                    # lhsT of all-ones broadcasts den to every partition for
                    # free, so no K=1 broadcast matmul / 1-lane reciprocal.
                    psd = ps2.tile([P, BLK], F32, name=f"psd{ic}", tag="psd", bufs=1,
                                   space="PSUM")
                    for jp in range(NPR):
                        ep = att.tile([P, 2, BLK], F8, name=f"ep{ic}_{jp}", tag="ep",
                                      bufs=4)
                        for jj in range(2):
                            j = 2 * jp + jj
                            pss = ps2.tile([P, BLK], F32, name=f"pss{ic}_{j}",
                                           tag="pss", bufs=3, space="PSUM")
                            for t in range(2):
                                nc.tensor.matmul(
                                    pss, k_sb[:, 2 * t:2 * t + 2, j * P:(j + 1) * P],
                                    q_sb[:, 2 * t:2 * t + 2,
                                         ic * BLK:(ic + 1) * BLK],
                                    start=(t == 0), stop=(t == 1), perf_mode=DR)
                            nc.scalar.activation(out=ep[:, jj, :], in_=pss,
                                                 func=AF.Exp, bias=expb_sb,
                                                 scale=SCALE)
                        for ct in range(CO):
                            nc.tensor.matmul(
                                pso[ct],
                                vT_sb[:, 2 * jp:2 * jp + 2, ct * P:(ct + 1) * P],
                                ep, start=(jp == 0), stop=(jp == NPR - 1),
                                perf_mode=DR)
                        nc.tensor.matmul(psd, ones8, ep, start=(jp == 0),
                                         stop=(jp == NPR - 1), perf_mode=DR)
                    return pso, psd

                def emit_copies(ic, pso, psd, last=False):
                    # Mid-pipeline: the ~3.4us reciprocal runs FIRST on DVE,
                    # during the next chunk's score loop (DVE is idle there);
                    # psd's single bank is recycled once it has been read.
                    # For the LAST chunk nothing hides it, so the pso copies
                    # go first — they gate the projection matmuls.
                    rbc = att.tile([P, BLK], F32, name=f"rbc{ic}", tag="rbc", bufs=2)
                    osc = att.tile([P, CO, BLK], F8, name=f"osc{ic}", tag="osc",
                                   bufs=2)

                    def emit_recip():
                        nc.vector.reciprocal(out=rbc, in_=psd)

                    def emit_osc():
                        # free the pso bank ring: 2 on DVE, 2 on ScalarE (all
                        # scaled 1/16 for fp8 range; undone by wo's x16).
                        nc.vector.tensor_scalar_mul(osc[:, 0, :], pso[0], IWS)
                        nc.scalar.activation(out=osc[:, 1, :], in_=pso[1],
                                             func=AF.Copy, scale=IWS)
                        nc.vector.tensor_scalar_mul(osc[:, 2, :], pso[2], IWS)
                        nc.scalar.activation(out=osc[:, 3, :], in_=pso[3],
                                             func=AF.Copy, scale=IWS)

                    if last:
                        emit_osc()
                        emit_recip()
                    else:
                        emit_recip()
                        emit_osc()
                    ypre = att.tile([P, CO, BLK], F32, name=f"ypre{ic}", tag="ypre",
                                    bufs=2)
                    for dt_ in range(CO):
                        nc.vector.tensor_scalar_add(
                            ypre[:, dt_, :], x_sb[:, dt_, ic * BLK:(ic + 1) * BLK],
                            vecs_sb[:, 8 + dt_:9 + dt_])
                    return osc, ypre, rbc

                def emit_tail(ic, osc, ypre, rbc):
                    # osc carries 1/16 and wo carries x16 -> psy is exactly
                    # sum(exp' * v) per channel; scale by 1/den' elementwise.
                    for dt_ in range(CO):
                        psy = ps2.tile([P, BLK], F32, name=f"psy{ic}_{dt_}", tag="pss",
                                       bufs=3, space="PSUM")
                        for t in range(2):
                            nc.tensor.matmul(
                                psy, wo_sb[:, 2 * t:2 * t + 2, dt_ * P:(dt_ + 1) * P],
                                osc[:, 2 * t:2 * t + 2, :], start=(t == 0),
                                stop=(t == 1), perf_mode=DR)
                        y = att.tile([P, BLK], F32, name=f"y{ic}_{dt_}", tag="y",
                                     bufs=2)
                        nc.vector.tensor_mul(out=y, in0=psy, in1=rbc)
                        nc.vector.tensor_add(out=y, in0=y, in1=ypre[:, dt_, :])
                        nc.sync.dma_start(out=out_r[:, dt_, ic * BLK:(ic + 1) * BLK],
                                          in_=y)

                prev = None
                for ic in range(NIC):
                    if prev is not None:
                        tail_args = emit_copies(prev[0], prev[1], prev[2])
                    cur = (ic, *emit_jloop(ic))
                    if prev is not None:
                        emit_tail(prev[0], *tail_args)
                    prev = cur
                tail_args = emit_copies(prev[0], prev[1], prev[2], last=True)
                emit_tail(prev[0], *tail_args)

    nc.compile()
    return nc


def _to_f8(a):
    return np.ascontiguousarray(np.asarray(a, np.float32)).astype(F8NP)


def _make_in_maps(inputs):
    x = np.asarray(inputs["x"], np.float32).reshape(B, C, HW)
    rep = {
        "wq": _to_f8(np.asarray(inputs["wq"], np.float32) * WS),
        "wk": _to_f8(np.asarray(inputs["wk"], np.float32) * WS),
        "wv": _to_f8(inputs["wv"]),
        "wo": _to_f8(np.asarray(inputs["wo"], np.float32) * WS),
        "bq": np.asarray(inputs["bq"], np.float32),
        "bk": np.asarray(inputs["bk"], np.float32),
        "bv": np.asarray(inputs["bv"], np.float32),
        "bo": np.asarray(inputs["bo"], np.float32),
        "gsc": np.asarray(inputs["gn_scale"], np.float32),
        "gbi": np.asarray(inputs["gn_bias"], np.float32),
    }
    emat = np.zeros((C, G), np.float32)
    emat[np.arange(C), np.arange(C) // GSZ] = 1.0
    # emat2[p, t*G+g] = emat[t*P+p, g]; etmat[g, t*P+c] = emat[t*P+c, g]
    rep["emat2"] = np.ascontiguousarray(
        emat.reshape(CO, P, G).transpose(1, 0, 2).reshape(P, CO * G))
    rep["etmat"] = np.ascontiguousarray(emat.T)
    vecs = np.zeros((P, 20), np.float32)
    for i, nm in enumerate(("bq", "bk", "bo", "gsc", "gbi")):
        vecs[:, 4 * i:4 * i + 4] = rep[nm].reshape(CO, P).T
    rep["vecs"] = vecs
    rep["bvbc"] = np.ascontiguousarray(np.broadcast_to(rep["bv"], (P, C)))
    for nm in ("bq", "bk", "bo", "gsc", "gbi", "bv"):
        del rep[nm]
    in_maps = []
    for core in range(8):
        b, half = core // 2, core % 2
        xb = x[b]
        own = xb[:, half * OWN:(half + 1) * OWN]
        oth = xb[:, (1 - half) * OWN:(2 - half) * OWN]
        xp = np.ascontiguousarray(np.concatenate([own, oth], axis=1))
        in_maps.append({"xin": xp, **rep})
    return in_maps


def kernel(**inputs):
    global _CACHED_NC, _LAST
    from concourse.bass_utils import run_bass_kernel_spmd

    if _CACHED_NC is None:
        _CACHED_NC = _build()
    in_maps = _make_in_maps(inputs)
    res = run_bass_kernel_spmd(_CACHED_NC, in_maps, core_ids=list(range(8)))
    _LAST = res
    out = np.empty((B, C, HW), np.float32)
    for core in range(8):
        b, half = core // 2, core % 2
        out[b][:, half * OWN:(half + 1) * OWN] = res.results[core]["out"]
    return out.reshape(B, C, H, W)
